# revision 1
# baseline (speedup 1.0000x reference)
"""Trainium2 Bass kernel for nn_NodeDetector (masked-node GATv2 ensemble).

Algorithm: the reference vmaps a full 2-layer GATv2 over 256 "masked node"
variants, but variant v differs from the shared base computation in exactly
one input row (row v).  We compute the base graph once and apply sparse
incremental updates per variant:

  phase 0  dense projections -> XL/XR (base rows) and XLs/XRs (masked rows)
  phase 1  base GAT layer 1: per-dst softmax sums (num1/den1) + g1_base
  (a)      per variant v: "light" g1 updates at out-neighbors d of v
           (only edges v->d changed: closed-form num/den delta)
  (b)      per variant v: full recompute of g1 at node v
  (d)      layer 2 at dst v only: gather xl2 of in-neighbors (base / self /
           rare light rows), one softmax, project + tanh.

Attention softmaxes skip the per-dst max subtraction (mathematically
identical; logits are O(10) so fp32 exp is safe).  All gathers use
host-built index tables (edge_index is host data) via gpsimd indirect DMA.
Work is sharded 32 variants per core across 8 cores; phases 0/1 are
replicated per core.  No collectives.
"""

import numpy as np

import concourse.bass as bass
import concourse.mybir as mybir
import concourse.tile as tile
from concourse import bacc
from concourse.bass_utils import run_bass_kernel_spmd
from concourse.masks import make_identity

F32 = mybir.dt.float32
I32 = mybir.dt.int32
AF = mybir.ActivationFunctionType
OP = mybir.AluOpType
AX = mybir.AxisListType

N = 256          # nodes / variants
F = 128          # NUM_HEAD * C2
C2 = 64
NH = 2
NCORES = 8
VPC = N // NCORES   # variants per core = 32
JC = 4              # partition-split of each dst's in-edge list
NEG = 0.2           # leaky relu slope


# --------------------------------------------------------------------------
# Host-side table construction
# --------------------------------------------------------------------------

def _build_tables(edge_index):
    src = edge_index[0].astype(np.int64)
    dst = edge_index[1].astype(np.int64)
    E = src.shape[0]

    in_edges = [[] for _ in range(N)]
    for e in range(E):
        in_edges[dst[e]].append(e)
    max_in = max(len(l) for l in in_edges)
    SLOTS = -(-max_in // JC)            # in-edge slots per jc row
    out_by_src = [[] for _ in range(N)]
    for e in range(E):
        if dst[e] != src[e]:
            out_by_src[src[e]].append(int(dst[e]))
    light = []
    for v in range(N):
        cnt = {}
        for d in out_by_src[v]:
            cnt[d] = cnt.get(d, 0) + 1
        light.append(sorted(cnt.items()))
    max_light = max(len(l) for l in light)
    K2 = 4 * (-(-max_light // 4))       # light slots per variant, mult of 4
    KA = K2 * VPC // 128                # light slots per partition

    def wrap16(flat):
        """int16 idx layout for dma_gather: value for flat position i lives
        at [i % 16, i // 16], tiled to 128 partitions."""
        flat = np.asarray(flat)
        num = flat.shape[0]
        A = np.zeros((16, num // 16), np.int16)
        A[np.arange(num) % 16, np.arange(num) // 16] = flat.astype(np.int16)
        return np.ascontiguousarray(np.tile(A, (8, 1)))

    def wrapPK(idx_pk):
        """[128, K] logical idx (out[p, k] = tab[idx_pk[p,k]]) -> wrapped."""
        return wrap16(idx_pk.T.reshape(-1))

    shared = {}
    IDX_P1 = np.zeros((N * JC, SLOTS), np.int32)
    MSK_P1 = np.zeros((N * JC, SLOTS), np.float32)
    IDXD_P1 = np.zeros((N * JC, 1), np.int32)
    for d in range(N):
        el = in_edges[d]
        for jc in range(JC):
            g = d * JC + jc
            IDXD_P1[g, 0] = d
            for s in range(SLOTS):
                k = jc * SLOTS + s
                if k < len(el):
                    IDX_P1[g, s] = src[el[k]]
                    MSK_P1[g, s] = 1.0
    # wrapped gather indices per half: out[p, t*SLOTS+s] = IDX_P1[512h+128t+p, s]
    IDX_P1W = np.zeros((2, 128, 4 * SLOTS * 128 // 16), np.int16)
    for h in range(2):
        pk = np.zeros((128, 4 * SLOTS), np.int64)
        for p in range(128):
            for t in range(4):
                pk[p, t * SLOTS:(t + 1) * SLOTS] = IDX_P1[512 * h + 128 * t + p]
        IDX_P1W[h] = wrapPK(pk)
    shared["IDX_P1W"] = IDX_P1W
    shared["MSK_P1"] = MSK_P1
    shared["IDXD_P1"] = IDXD_P1

    # combine matrices, [128, 4, 128]: input tile t', partial row p -> col
    CMB4 = np.zeros((128, 4, 128), np.float32)
    for tp in range(4):
        for p in range(128):
            CMB4[p, tp, 32 * tp + p // 4] = 1.0
    shared["CMB4"] = CMB4

    percore = []
    for c in range(NCORES):
        t = {}
        V = list(range(c * VPC, (c + 1) * VPC))
        IDX_A_T1 = np.zeros((128, KA), np.int32)
        IDX_A_V = np.zeros((128, KA), np.int32)
        IDX_A_VS = np.zeros((128, KA), np.int32)
        C_A = np.zeros((128, KA), np.float32)
        for r in range(128 * KA):
            vi, slot = divmod(r, K2)
            p, k = divmod(r, KA)
            v = V[vi]
            IDX_A_V[p, k] = v
            IDX_A_VS[p, k] = 256 + v
            if slot < len(light[v]):
                d, cc = light[v][slot]
                IDX_A_T1[p, k] = d
                C_A[p, k] = float(cc)
        t["IDX_A_T1W"] = wrapPK(IDX_A_T1)
        t["IDX_A_VW"] = wrapPK(IDX_A_V)
        t["IDX_A_VSW"] = wrapPK(IDX_A_VS)
        t["C_A"] = C_A

        IDX_B_XL = np.zeros((128, SLOTS), np.int32)
        MSK_B = np.zeros((128, SLOTS), np.float32)
        IDX_B_V = np.zeros((128, 1), np.int32)
        for vi, v in enumerate(V):
            el = in_edges[v]
            for jc in range(JC):
                p = vi * JC + jc
                IDX_B_V[p, 0] = v
                for s in range(SLOTS):
                    k = jc * SLOTS + s
                    if k < len(el):
                        sn = int(src[el[k]])
                        IDX_B_XL[p, s] = 256 + v if sn == v else sn
                        MSK_B[p, s] = 1.0
        t["IDX_B_XLW"] = wrapPK(IDX_B_XL)
        t["MSK_B"] = MSK_B
        t["IDX_B_V"] = IDX_B_V

        IDX_D_T2 = np.zeros((128, SLOTS), np.int32)
        MSK_D = np.zeros((128, SLOTS), np.float32)
        IDX_D_V = np.zeros((128, 1), np.int32)
        IDX_RARE = np.zeros((128, 1), np.int32)
        rare_map = {}
        for vi, v in enumerate(V):
            lpos = {d: i for i, (d, _) in enumerate(light[v])}
            el = in_edges[v]
            for jc in range(JC):
                p = vi * JC + jc
                IDX_D_V[p, 0] = vi
                for s in range(SLOTS):
                    k = jc * SLOTS + s
                    if k < len(el):
                        sn = int(src[el[k]])
                        MSK_D[p, s] = 1.0
                        if sn == v:
                            idx = 256 + vi
                        elif sn in lpos:
                            key = (vi, sn)
                            if key not in rare_map:
                                rs = len(rare_map)
                                assert rs < 128, "rare-row overflow"
                                rare_map[key] = rs
                                IDX_RARE[rs, 0] = vi * K2 + lpos[sn]
                            idx = 288 + rare_map[key]
                        else:
                            idx = sn
                        IDX_D_T2[p, s] = idx
        t["IDX_D_T2W"] = wrapPK(IDX_D_T2)
        t["MSK_D"] = MSK_D
        t["IDX_D_V"] = IDX_D_V
        t["IDX_RARE"] = IDX_RARE
        percore.append(t)

    dims = dict(SLOTS=SLOTS, K2=K2, KA=KA)
    return shared, percore, dims


# --------------------------------------------------------------------------
# Device program
# --------------------------------------------------------------------------

def _build_program(dims, lrelu_act=True, dbg=False):
    SLOTS, K2, KA = dims["SLOTS"], dims["K2"], dims["KA"]

    nc = bacc.Bacc("TRN2", target_bir_lowering=False, debug=False)

    def inp(name, shape, dtype=F32):
        return nc.dram_tensor(name, list(shape), dtype, kind="ExternalInput")

    D = {}
    D["x"] = inp("x", [N, 64])
    D["E_emb"] = inp("E_emb", [N, 64])
    for nm, sh in [("node_proj", [64, 128]), ("emb_proj", [64, 128]),
                   ("conv_w0", [128, 128]), ("conv_w1", [128, 128]),
                   ("conv_b", [128, 1]), ("lin2_w", [128, 64]),
                   ("lin2_b", [64, 1]), ("masked_proj", [64, 64]),
                   ("normal_proj", [64, 64]), ("g1_wl", [64, 128]),
                   ("g1_bl", [128, 1]), ("g1_wr", [64, 128]),
                   ("g1_br", [128, 1]), ("g2_wl", [64, 128]),
                   ("g2_wr", [64, 128]), ("rec_w", [64, 64]),
                   ("rec_b", [64, 1]), ("att1_rep", [128, 128]),
                   ("att2_rep", [128, 128]), ("g1bias_rep", [128, 64]),
                   ("g2bias_rep", [128, 64]), ("blr_rep", [128, 128]),
                   ("CMB4", [128, 4, 128])]:
        D[nm] = inp(nm, sh)
    for nm, sh in [("IDXD_P1", [N * JC, 1]), ("IDX_B_V", [128, 1]),
                   ("IDX_D_V", [128, 1]), ("IDX_RARE", [128, 1])]:
        D[nm] = inp(nm, sh, I32)
    I16 = mybir.dt.int16
    for nm, sh in [("IDX_P1W", [2, 128, 4 * SLOTS * 8]),
                   ("IDX_A_T1W", [128, KA * 8]),
                   ("IDX_A_VW", [128, KA * 8]),
                   ("IDX_A_VSW", [128, KA * 8]),
                   ("IDX_B_XLW", [128, SLOTS * 8]),
                   ("IDX_D_T2W", [128, SLOTS * 8])]:
        D[nm] = inp(nm, sh, I16)
    for nm, sh in [("MSK_P1", [N * JC, SLOTS]), ("C_A", [128, KA]),
                   ("MSK_B", [128, SLOTS]), ("MSK_D", [128, SLOTS])]:
        D[nm] = inp(nm, sh)

    D["out"] = nc.dram_tensor("out", [VPC, 64], F32, kind="ExternalOutput")
    D["XLcat"] = nc.dram_tensor("XLcat", [2 * N, F], F32)
    D["XRtab"] = nc.dram_tensor("XRtab", [N, F], F32)
    D["XRStab"] = nc.dram_tensor("XRStab", [N, F], F32)
    D["T1"] = nc.dram_tensor("T1", [N, 320], F32)
    D["G1L"] = nc.dram_tensor("G1L", [VPC * K2, C2], F32)
    D["T2"] = nc.dram_tensor("T2", [N + VPC + 128, F], F32)
    D["XR2S"] = nc.dram_tensor("XR2S", [VPC, F], F32)

    with tile.TileContext(nc) as tc:
        _trace(nc, tc, D, SLOTS, K2, KA, lrelu_act, dbg)
    nc.compile()
    return nc


def _trace(nc, tc, D, SLOTS, K2, KA, lrelu_act=True, dbg=False):
    import contextlib
    ctx = contextlib.ExitStack()
    with ctx:
        consts = ctx.enter_context(tc.tile_pool(name="consts", bufs=1))
        small = ctx.enter_context(tc.tile_pool(name="small", bufs=1))
        big = ctx.enter_context(tc.tile_pool(name="big", bufs=1))
        psum = ctx.enter_context(tc.tile_pool(name="psum", bufs=4,
                                              space="PSUM"))
        psum_acc = ctx.enter_context(tc.tile_pool(name="psacc", bufs=2,
                                                  space="PSUM"))

        dma = nc.sync.dma_start
        I16 = mybir.dt.int16

        def dgather(out_ap, in_ap, idx_ap, num, elem):
            nc.gpsimd.dma_gather(out_ap=out_ap, in_ap=in_ap, idxs_ap=idx_ap,
                                 num_idxs=num, num_idxs_reg=num,
                                 elem_size=elem, single_packet=False)
        def dbg_dump(name, ap, dtype=F32):
            if not dbg:
                return
            sh = list(ap.shape)
            t_ = nc.dram_tensor("dbg_" + name, sh, dtype,
                                kind="ExternalOutput")
            dma(out=t_[:], in_=ap)
        tt = nc.vector.tensor_tensor
        red = nc.vector.tensor_reduce
        act = nc.scalar.activation
        gather = nc.gpsimd.indirect_dma_start
        IOA = bass.IndirectOffsetOnAxis

        # ---------------- constants ----------------
        ident = consts.tile([128, 128], F32, tag="ident")
        make_identity(nc, ident[:])

        def load(name, shape, dtype=F32):
            t_ = consts.tile(list(shape), dtype, tag="c_" + name)
            dma(out=t_[:], in_=D[name][:])
            return t_

        w_node = load("node_proj", [64, 128])
        w_emb = load("emb_proj", [64, 128])
        w_c0 = load("conv_w0", [128, 128])
        w_c1 = load("conv_w1", [128, 128])
        b_conv = load("conv_b", [128, 1])
        w_lin2 = load("lin2_w", [128, 64])
        b_lin2 = load("lin2_b", [64, 1])
        w_mask = load("masked_proj", [64, 64])
        w_norm = load("normal_proj", [64, 64])
        w_1l = load("g1_wl", [64, 128])
        b_1l = load("g1_bl", [128, 1])
        w_1r = load("g1_wr", [64, 128])
        b_1r = load("g1_br", [128, 1])
        w_2l = load("g2_wl", [64, 128])
        w_2r = load("g2_wr", [64, 128])
        w_rec = load("rec_w", [64, 64])
        b_rec = load("rec_b", [64, 1])
        att1 = load("att1_rep", [128, 128])
        att2 = load("att2_rep", [128, 128])
        g1bias = load("g1bias_rep", [128, 64])
        g2bias = load("g2bias_rep", [128, 64])
        blr = load("blr_rep", [128, 128])
        cmb4 = load("CMB4", [128, 4, 128])

        # ---------------- helpers ----------------
        def ts_mul(out, in0, s):
            nc.vector.tensor_scalar_mul(out=out, in0=in0, scalar1=s)

        def lrelu(flat_ap, nfree, tag):
            if lrelu_act:
                act(out=flat_ap, in_=flat_ap, func=AF.Lrelu, alpha=NEG)
            else:
                t_ = big.tile([128, nfree], F32, tag="lr_" + tag)
                ta = t_[:flat_ap.shape[0], :]
                ts_mul(ta, flat_ap, NEG)
                tt(out=flat_ap, in0=flat_ap, in1=ta, op=OP.max)

        def elu_inplace(x_ap, scratch_pool, nfree, tag):
            xpos = scratch_pool.tile([128, nfree], F32, tag=tag + "_xp")
            nrow = x_ap.shape[0]
            xp = xpos[:nrow, :]
            nc.vector.tensor_scalar_max(out=xp, in0=x_ap, scalar1=0.0)
            nc.vector.tensor_scalar_min(out=x_ap, in0=x_ap, scalar1=0.0)
            act(out=x_ap, in_=x_ap, func=AF.Exp)
            nc.vector.tensor_scalar_add(out=x_ap, in0=x_ap, scalar1=-1.0)
            nc.vector.tensor_add(out=x_ap, in0=x_ap, in1=xp)
            return x_ap

        def head_mean_bias_elu(nd_ap, nrow, bias_rep, tag):
            """nd_ap [nrow, F+NH] = (num|den) -> elu(mean_h(num/den)+bias)."""
            rec = small.tile([128, NH], F32, tag=tag + "_rec")
            nc.vector.reciprocal(out=rec[:nrow, :], in_=nd_ap[:, F:F + NH])
            r0 = small.tile([128, C2], F32, tag=tag + "_r0")
            r1 = small.tile([128, C2], F32, tag=tag + "_r1")
            ts_mul(r0[:nrow, :], nd_ap[:, 0:C2], rec[:nrow, 0:1])
            ts_mul(r1[:nrow, :], nd_ap[:, C2:F], rec[:nrow, 1:2])
            tt(out=r0[:nrow, :], in0=r0[:nrow, :], in1=r1[:nrow, :], op=OP.add)
            ts_mul(r0[:nrow, :], r0[:nrow, :], 0.5)
            tt(out=r0[:nrow, :], in0=r0[:nrow, :], in1=bias_rep[:nrow, :],
               op=OP.add)
            return elu_inplace(r0[:nrow, :], small, C2, tag)

        # ---------------- phase 0 ----------------
        def mm_to_sbuf(lhsT, rhs, M, Nf, tag, bias=None, func=AF.Identity,
                       extra=None):
            out_tile = small.tile([M, Nf], F32, tag=tag)
            ps = psum.tile([128, 256], F32, tag="ps")
            nc.tensor.matmul(ps[:M, :Nf], lhsT, rhs, start=True,
                             stop=extra is None)
            if extra is not None:
                nc.tensor.matmul(ps[:M, :Nf], extra[0], extra[1],
                                 start=False, stop=True)
            if bias is None:
                act(out=out_tile[:], in_=ps[:M, :Nf], func=func)
            else:
                act(out=out_tile[:], in_=ps[:M, :Nf], func=func, bias=bias)
            return out_tile

        xT = small.tile([64, 256], F32, tag="xT")
        eT = small.tile([64, 256], F32, tag="eT")
        for h in range(2):
            for (dname, dstT, tg) in ((("x"), xT, "ldx"), ("E_emb", eT, "lde")):
                tin = small.tile([128, 64], F32, tag="ph0_" + tg)
                dma(out=tin[:], in_=D[dname][128 * h:128 * (h + 1), :])
                pst = psum.tile([64, 128], F32, tag="ps")
                nc.tensor.transpose(pst[:], tin[:], ident[:])
                nc.vector.tensor_copy(out=dstT[:, 128 * h:128 * (h + 1)],
                                      in_=pst[:])

        xpT = mm_to_sbuf(w_node[:], xT[:], 128, 256, "xpT")
        epT = mm_to_sbuf(w_emb[:], eT[:], 128, 256, "epT")
        HbT = mm_to_sbuf(w_c0[:], epT[:], 128, 256, "HbT", bias=b_conv[:],
                         func=AF.Tanh, extra=(w_c1[:], xpT[:]))
        HsT = mm_to_sbuf(w_c0[:], epT[:], 128, 256, "HsT", bias=b_conv[:],
                         func=AF.Tanh)
        MbT = mm_to_sbuf(w_lin2[:], HbT[:], 64, 256, "MbT", bias=b_lin2[:])
        MsT = mm_to_sbuf(w_lin2[:], HsT[:], 64, 256, "MsT", bias=b_lin2[:])
        PbT = mm_to_sbuf(w_norm[:], MbT[:], 64, 256, "PbT")
        PsT = mm_to_sbuf(w_mask[:], MsT[:], 64, 256, "PsT")
        XLT = mm_to_sbuf(w_1l[:], PbT[:], 128, 256, "XLT", bias=b_1l[:])
        XRT = mm_to_sbuf(w_1r[:], PbT[:], 128, 256, "XRT", bias=b_1r[:])
        XLsT = mm_to_sbuf(w_1l[:], PsT[:], 128, 256, "XLsT", bias=b_1l[:])
        XRsT = mm_to_sbuf(w_1r[:], PsT[:], 128, 256, "XRsT", bias=b_1r[:])

        def store_nodemajor(srcT, dram_ap_fn, tag):
            for h in range(2):
                ps = psum.tile([128, 128], F32, tag="ps")
                nc.tensor.transpose(ps[:], srcT[:, 128 * h:128 * (h + 1)],
                                    ident[:])
                sb = small.tile([128, 128], F32, tag="nm_sb_" + tag)
                nc.vector.tensor_copy(out=sb[:], in_=ps[:])
                dma(out=dram_ap_fn(h), in_=sb[:])

        store_nodemajor(XLT, lambda h: D["XLcat"][128 * h:128 * (h + 1), :],
                        "xl")
        store_nodemajor(XLsT,
                        lambda h: D["XLcat"][N + 128 * h:N + 128 * (h + 1), :],
                        "xls")
        store_nodemajor(XRT, lambda h: D["XRtab"][128 * h:128 * (h + 1), :],
                        "xr")
        store_nodemajor(XRT, lambda h: D["T1"][128 * h:128 * (h + 1), 0:F],
                        "xrt1")
        store_nodemajor(XRsT, lambda h: D["XRStab"][128 * h:128 * (h + 1), :],
                        "xrs")
        zpad = small.tile([128, 62], F32, tag="zpad")
        nc.vector.memset(zpad[:], 0.0)
        for h in range(2):
            dma(out=D["T1"][128 * h:128 * (h + 1), 258:320], in_=zpad[:])

        # ---------------- shared GAT edge stage ----------------
        def edge_stage(xlg_tile, nslot, mask_tile, att, xr_tile, tagp):
            """xlg_tile [128, nslot*F] gathered xl rows (consumed -> w*xl).
            xr_tile [128, 1, F]; returns w tile [128, nslot, NH]."""
            xlg3 = xlg_tile[:].rearrange("p (s f) -> p s f", s=nslot)
            u = big.tile([128, nslot * F], F32, tag=tagp + "_u")
            u3 = u[:].rearrange("p (s f) -> p s f", s=nslot)
            tt(out=u3, in0=xlg3,
               in1=xr_tile[:].rearrange("p f -> p () f")
               .to_broadcast([128, nslot, F]), op=OP.add)
            lrelu(u[:], nslot * F, tagp + "_u")
            attb = att[:].rearrange("p (h f) -> p () h f", h=NH) \
                .to_broadcast([128, nslot, NH, C2])
            u4 = u[:].rearrange("p (s h f) -> p s h f", s=nslot, h=NH)
            tt(out=u4, in0=u4, in1=attb, op=OP.mult)
            lg = small.tile([128, nslot, NH], F32, tag=tagp + "_lg")
            red(out=lg[:], in_=u4, axis=AX.X, op=OP.add)
            act(out=lg[:], in_=lg[:], func=AF.Exp)
            mb = mask_tile[:].rearrange("p s -> p s ()") \
                .to_broadcast([128, nslot, NH])
            tt(out=lg[:], in0=lg[:], in1=mb, op=OP.mult)
            wb = lg[:].rearrange("p s h -> p s h ()") \
                .to_broadcast([128, nslot, NH, C2])
            xlg4 = xlg_tile[:].rearrange("p (s h f) -> p s h f", s=nslot,
                                         h=NH)
            tt(out=xlg4, in0=xlg4, in1=wb, op=OP.mult)
            return lg

        def softmax_combine(xlg_tile, lg, nslot, tagp):
            comb = small.tile([128, F + NH], F32, tag=tagp + "_comb")
            red(out=comb[:, 0:F],
                in_=xlg_tile[:].rearrange("p (s f) -> p f s", s=nslot),
                axis=AX.X, op=OP.add)
            red(out=comb[:, F:F + NH],
                in_=lg[:].rearrange("p s h -> p h s"),
                axis=AX.X, op=OP.add)
            return comb

        # ---------------- phase 1: base GAT layer 1 ----------------
        g1b_chunks = []
        for h in range(2):
            idx = small.tile([128, 4 * SLOTS * 8], I16, tag="p1_idx")
            dma(out=idx[:], in_=D["IDX_P1W"][h, :, :])
            msk = small.tile([128, 4, SLOTS], F32, tag="p1_msk")
            dma(out=msk[:], in_=D["MSK_P1"][512 * h:512 * (h + 1), :]
                .rearrange("(t p) s -> p t s", p=128))
            idxd = small.tile([128, 4, 1], I32, tag="p1_idxd")
            dma(out=idxd[:], in_=D["IDXD_P1"][512 * h:512 * (h + 1), :]
                .rearrange("(t p) s -> p t s", p=128))

            xlg = big.tile([128, 4 * SLOTS * F], F32, tag="p1_xlg")
            dgather(xlg[:].rearrange("p (k f) -> p k f", k=4 * SLOTS),
                    D["XLcat"][:], idx[:], 4 * SLOTS * 128, F)
            xrr = big.tile([128, 4, F], F32, tag="p1_xrr")
            for tpi in range(4):
                gather(out=xrr[:, tpi, :], out_offset=None, in_=D["XRtab"][:],
                       in_offset=IOA(ap=idxd[:, tpi, :], axis=0))
            dbg_dump("p1_xlg%d" % h, xlg[:])
            dbg_dump("p1_xrr%d" % h, xrr[:])

            xlg4 = xlg[:].rearrange("p (t s f) -> p t s f", t=4, s=SLOTS)
            u = big.tile([128, 4 * SLOTS * F], F32, tag="p1_u")
            u4 = u[:].rearrange("p (t s f) -> p t s f", t=4, s=SLOTS)
            tt(out=u4, in0=xlg4,
               in1=xrr[:].rearrange("p t f -> p t () f").to_broadcast([128, 4, SLOTS, F]), op=OP.add)
            lrelu(u[:], 4 * SLOTS * F, "p1_u")
            attb = att1[:].rearrange("p (h f) -> p () () h f", h=NH) \
                .to_broadcast([128, 4, SLOTS, NH, C2])
            u5 = u[:].rearrange("p (t s h f) -> p t s h f", t=4, s=SLOTS,
                                h=NH)
            tt(out=u5, in0=u5, in1=attb, op=OP.mult)
            lg = small.tile([128, 4, SLOTS, NH], F32, tag="p1_lg")
            red(out=lg[:], in_=u5, axis=AX.X, op=OP.add)
            act(out=lg[:], in_=lg[:], func=AF.Exp)
            mb = msk[:].rearrange("p t s -> p t s ()") \
                .to_broadcast([128, 4, SLOTS, NH])
            tt(out=lg[:], in0=lg[:], in1=mb, op=OP.mult)
            dbg_dump("p1_lg%d" % h, lg[:])
            wb = lg[:].rearrange("p t s h -> p t s h ()") \
                .to_broadcast([128, 4, SLOTS, NH, C2])
            xlg5 = xlg[:].rearrange("p (t s h f) -> p t s h f", t=4, s=SLOTS,
                                    h=NH)
            tt(out=xlg5, in0=xlg5, in1=wb, op=OP.mult)

            comb = small.tile([128, 4, F + NH], F32, tag="p1_comb")
            red(out=comb[:, :, 0:F],
                in_=xlg[:].rearrange("p (t s f) -> p t f s", t=4, s=SLOTS),
                axis=AX.X, op=OP.add)
            red(out=comb[:, :, F:F + NH],
                in_=lg[:].rearrange("p t s h -> p t h s"),
                axis=AX.X, op=OP.add)

            nd_ps = psum_acc.tile([128, F + NH], F32, tag="p1_ndps")
            for tp in range(4):
                nc.tensor.matmul(nd_ps[:], cmb4[:, tp, :], comb[:, tp, :],
                                 start=(tp == 0), stop=(tp == 3))
            nd = small.tile([128, F + NH], F32, tag="p1_nd")
            nc.vector.tensor_copy(out=nd[:], in_=nd_ps[:])
            dbg_dump("p1_comb%d" % h, comb[:])
            dbg_dump("p1_nd%d" % h, nd[:])
            dma(out=D["T1"][128 * h:128 * (h + 1), F:2 * F + NH], in_=nd[:])
            g1b = head_mean_bias_elu(nd[:], 128, g1bias, "p1g" + str(h))
            dbg_dump("g1b%d" % h, g1b)
            g1b_chunks.append(g1b)

        # g1_base^T -> XL2_base (T2 rows 0:256)
        g1bT = small.tile([64, 256], F32, tag="g1bT")
        for h in range(2):
            ps = psum.tile([64, 128], F32, tag="ps")
            nc.tensor.transpose(ps[:], g1b_chunks[h], ident[:])
            nc.vector.tensor_copy(out=g1bT[:, 128 * h:128 * (h + 1)],
                                  in_=ps[:])
        for h in range(2):
            ps = psum.tile([128, 128], F32, tag="ps")
            nc.tensor.matmul(ps[:], g1bT[:, 128 * h:128 * (h + 1)], w_2l[:],
                             start=True, stop=True)
            sb = small.tile([128, 128], F32, tag="p15_sb")
            nc.vector.tensor_copy(out=sb[:], in_=ps[:])
            dma(out=D["T2"][128 * h:128 * (h + 1), :], in_=sb[:])

        # ---------------- (b): full recompute of dst v ----------------
        idxb = small.tile([128, SLOTS * 8], I16, tag="b_idx")
        dma(out=idxb[:], in_=D["IDX_B_XLW"][:])
        mskb = small.tile([128, SLOTS], F32, tag="b_msk")
        dma(out=mskb[:], in_=D["MSK_B"][:])
        idxbv = small.tile([128, 1], I32, tag="b_idxv")
        dma(out=idxbv[:], in_=D["IDX_B_V"][:])
        xlgb = big.tile([128, SLOTS * F], F32, tag="b_xlg")
        dgather(xlgb[:].rearrange("p (k f) -> p k f", k=SLOTS),
                D["XLcat"][:], idxb[:], SLOTS * 128, F)
        dbg_dump("b_xlg", xlgb[:])
        xrrb = big.tile([128, F], F32, tag="b_xrr")
        gather(out=xrrb[:], out_offset=None, in_=D["XRStab"][:],
               in_offset=IOA(ap=idxbv[:], axis=0))
        dbg_dump("b_xrr", xrrb[:])
        lgb = edge_stage(xlgb, SLOTS, mskb, att1, xrrb, "b")
        dbg_dump("b_lg", lgb[:])
        combb = softmax_combine(xlgb, lgb, SLOTS, "b")
        dbg_dump("b_comb", combb[:])
        ndb_ps = psum.tile([VPC, F + NH], F32, tag="ps")
        nc.tensor.matmul(ndb_ps[:], cmb4[:, 0, 0:VPC], combb[:],
                         start=True, stop=True)
        ndb = small.tile([VPC, F + NH], F32, tag="b_nd")
        nc.vector.tensor_copy(out=ndb[:], in_=ndb_ps[:])
        g1self = head_mean_bias_elu(ndb[:], VPC, g1bias, "bg1")
        dbg_dump("g1self", g1self)

        ps_t = psum.tile([C2, VPC], F32, tag="ps")
        nc.tensor.transpose(ps_t[:], g1self, ident[:VPC, :VPC])
        g1sT = small.tile([C2, VPC], F32, tag="g1sT")
        nc.vector.tensor_copy(out=g1sT[:], in_=ps_t[:])
        ps_l = psum.tile([VPC, F], F32, tag="ps")
        nc.tensor.matmul(ps_l[:], g1sT[:], w_2l[:], start=True, stop=True)
        sb_l = small.tile([VPC, F], F32, tag="b_sbl")
        nc.vector.tensor_copy(out=sb_l[:], in_=ps_l[:])
        dma(out=D["T2"][N:N + VPC, :], in_=sb_l[:])
        ps_r = psum.tile([VPC, F], F32, tag="ps")
        nc.tensor.matmul(ps_r[:], g1sT[:], w_2r[:], start=True, stop=True)
        sb_r = small.tile([VPC, F], F32, tag="b_sbr")
        tt(out=sb_r[:], in0=ps_r[:], in1=blr[:VPC, :], op=OP.add)
        dma(out=D["XR2S"][:], in_=sb_r[:])

        # ---------------- (a): light dst updates ----------------
        idx_t1 = small.tile([128, KA * 8], I16, tag="a_it1")
        dma(out=idx_t1[:], in_=D["IDX_A_T1W"][:])
        idx_v = small.tile([128, KA * 8], I16, tag="a_iv")
        dma(out=idx_v[:], in_=D["IDX_A_VW"][:])
        idx_vs = small.tile([128, KA * 8], I16, tag="a_ivs")
        dma(out=idx_vs[:], in_=D["IDX_A_VSW"][:])
        ca = small.tile([128, KA], F32, tag="a_ca")
        dma(out=ca[:], in_=D["C_A"][:])

        t1g = big.tile([128, KA * 320], F32, tag="a_t1g")
        dgather(t1g[:].rearrange("p (k f) -> p k f", k=KA),
                D["T1"][:], idx_t1[:], KA * 128, 320)
        xlv = big.tile([128, KA * F], F32, tag="a_xlv")
        dgather(xlv[:].rearrange("p (k f) -> p k f", k=KA),
                D["XLcat"][:], idx_v[:], KA * 128, F)
        xlsv = big.tile([128, KA * F], F32, tag="a_xlsv")
        dgather(xlsv[:].rearrange("p (k f) -> p k f", k=KA),
                D["XLcat"][:], idx_vs[:], KA * 128, F)

        t1g3 = t1g[:].rearrange("p (k f) -> p k f", k=KA)  # f = 320
        t1xr = t1g3[:, :, 0:F]
        t1num = t1g3[:, :, F:2 * F]
        t1den = t1g3[:, :, 2 * F:2 * F + NH]
        xlv3 = xlv[:].rearrange("p (k f) -> p k f", k=KA)
        xlsv3 = xlsv[:].rearrange("p (k f) -> p k f", k=KA)

        def logits_expC(xl3, tg):
            u = big.tile([128, KA * F], F32, tag="a_u" + tg)
            u3 = u[:].rearrange("p (k f) -> p k f", k=KA)
            tt(out=u3, in0=xl3, in1=t1xr, op=OP.add)
            lrelu(u[:], KA * F, "a_u" + tg)
            attb = att1[:].rearrange("p (h f) -> p () h f", h=NH) \
                .to_broadcast([128, KA, NH, C2])
            u4 = u[:].rearrange("p (k h f) -> p k h f", k=KA, h=NH)
            tt(out=u4, in0=u4, in1=attb, op=OP.mult)
            lw = small.tile([128, KA, NH], F32, tag="a_lw" + tg)
            red(out=lw[:], in_=u4, axis=AX.X, op=OP.add)
            act(out=lw[:], in_=lw[:], func=AF.Exp)
            cb = ca[:].rearrange("p k -> p k ()").to_broadcast([128, KA, NH])
            tt(out=lw[:], in0=lw[:], in1=cb, op=OP.mult)
            return lw

        dbg_dump("a_t1g", t1g[:])
        dbg_dump("a_xlv", xlv[:])
        dbg_dump("a_xlsv", xlsv[:])
        wn = logits_expC(xlsv3, "n")    # C * w_new
        wo = logits_expC(xlv3, "o")     # C * w_old
        dbg_dump("a_wn", wn[:])
        dbg_dump("a_wo", wo[:])

        dden = small.tile([128, KA, NH], F32, tag="a_dden")
        tt(out=dden[:], in0=wn[:], in1=wo[:], op=OP.subtract)
        tt(out=dden[:], in0=dden[:], in1=t1den, op=OP.add)
        wnb = wn[:].rearrange("p k h -> p k h ()") \
            .to_broadcast([128, KA, NH, C2])
        xlsv4 = xlsv[:].rearrange("p (k h f) -> p k h f", k=KA, h=NH)
        tt(out=xlsv4, in0=xlsv4, in1=wnb, op=OP.mult)
        wob = wo[:].rearrange("p k h -> p k h ()") \
            .to_broadcast([128, KA, NH, C2])
        xlv4 = xlv[:].rearrange("p (k h f) -> p k h f", k=KA, h=NH)
        tt(out=xlv4, in0=xlv4, in1=wob, op=OP.mult)
        tt(out=xlsv3, in0=xlsv3, in1=xlv3, op=OP.subtract)
        tt(out=xlsv3, in0=xlsv3, in1=t1num, op=OP.add)
        nc.vector.reciprocal(out=dden[:], in_=dden[:])
        ddb = dden[:].rearrange("p k h -> p k h ()") \
            .to_broadcast([128, KA, NH, C2])
        tt(out=xlsv4, in0=xlsv4, in1=ddb, op=OP.mult)
        radd = big.tile([128, KA, C2], F32, tag="a_radd")
        tt(out=radd[:], in0=xlsv4[:, :, 0, :], in1=xlsv4[:, :, 1, :],
           op=OP.add)
        ts_mul(radd[:], radd[:], 0.5)
        g1bb = g1bias[:].rearrange("p f -> p () f").to_broadcast(
            [128, KA, C2])
        tt(out=radd[:], in0=radd[:], in1=g1bb, op=OP.add)
        radd_flat = radd[:].rearrange("p k f -> p (k f)")
        elu_inplace(radd_flat, big, KA * C2, "a_elu")
        dma(out=D["G1L"][:].rearrange("(p k) f -> p k f", p=128), in_=radd[:])

        # ---------------- rare light rows -> T2 rows 288: ----------------
        idx_r = small.tile([128, 1], I32, tag="r_idx")
        dma(out=idx_r[:], in_=D["IDX_RARE"][:])
        grare = small.tile([128, C2], F32, tag="r_g")
        gather(out=grare[:], out_offset=None, in_=D["G1L"][:],
               in_offset=IOA(ap=idx_r[:], axis=0))
        ps_rt = psum.tile([C2, 128], F32, tag="ps")
        nc.tensor.transpose(ps_rt[:], grare[:], ident[:])
        grT = small.tile([C2, 128], F32, tag="grT")
        nc.vector.tensor_copy(out=grT[:], in_=ps_rt[:])
        ps_rm = psum.tile([128, F], F32, tag="ps")
        nc.tensor.matmul(ps_rm[:], grT[:], w_2l[:], start=True, stop=True)
        sb_rm = small.tile([128, F], F32, tag="r_sb")
        nc.vector.tensor_copy(out=sb_rm[:], in_=ps_rm[:])
        dma(out=D["T2"][N + VPC:N + VPC + 128, :], in_=sb_rm[:])

        # ---------------- (d): layer 2 at dst v ----------------
        idxd2 = small.tile([128, SLOTS * 8], I16, tag="d_idx")
        dma(out=idxd2[:], in_=D["IDX_D_T2W"][:])
        mskd = small.tile([128, SLOTS], F32, tag="d_msk")
        dma(out=mskd[:], in_=D["MSK_D"][:])
        idxdv = small.tile([128, 1], I32, tag="d_idxv")
        dma(out=idxdv[:], in_=D["IDX_D_V"][:])
        xl2g = big.tile([128, SLOTS * F], F32, tag="d_xlg")
        dgather(xl2g[:].rearrange("p (k f) -> p k f", k=SLOTS),
                D["T2"][:], idxd2[:], SLOTS * 128, F)
        xr2r = big.tile([128, F], F32, tag="d_xrr")
        gather(out=xr2r[:], out_offset=None, in_=D["XR2S"][:],
               in_offset=IOA(ap=idxdv[:], axis=0))
        dbg_dump("d_xlg", xl2g[:])
        dbg_dump("d_xrr", xr2r[:])
        lgd = edge_stage(xl2g, SLOTS, mskd, att2, xr2r, "d")
        dbg_dump("d_lg", lgd[:])
        combd = softmax_combine(xl2g, lgd, SLOTS, "d")
        ndd_ps = psum.tile([VPC, F + NH], F32, tag="ps")
        nc.tensor.matmul(ndd_ps[:], cmb4[:, 0, 0:VPC], combd[:],
                         start=True, stop=True)
        ndd = small.tile([VPC, F + NH], F32, tag="d_nd")
        nc.vector.tensor_copy(out=ndd[:], in_=ndd_ps[:])
        g2row = head_mean_bias_elu(ndd[:], VPC, g2bias, "dg2")
        dbg_dump("g2row", g2row)

        # out = tanh(g2row @ rec_w + rec_b)
        ps_ot = psum.tile([C2, VPC], F32, tag="ps")
        nc.tensor.transpose(ps_ot[:], g2row, ident[:VPC, :VPC])
        g2T = small.tile([C2, VPC], F32, tag="g2T")
        nc.vector.tensor_copy(out=g2T[:], in_=ps_ot[:])
        ps_om = psum.tile([C2, VPC], F32, tag="ps")
        nc.tensor.matmul(ps_om[:], w_rec[:], g2T[:], start=True, stop=True)
        outT = small.tile([C2, VPC], F32, tag="outT")
        act(out=outT[:], in_=ps_om[:], func=AF.Tanh, bias=b_rec[:])
        ps_of = psum.tile([VPC, C2], F32, tag="ps")
        nc.tensor.transpose(ps_of[:], outT[:], ident[:C2, :C2])
        outsb = small.tile([VPC, C2], F32, tag="outsb")
        nc.vector.tensor_copy(out=outsb[:], in_=ps_of[:])
        dma(out=D["out"][:], in_=outsb[:])
        if dbg:
            dbg_dump("XLcat", D["XLcat"][:])
            dbg_dump("T1", D["T1"][:])
            dbg_dump("T2", D["T2"][:])
            dbg_dump("XR2S", D["XR2S"][:])
            dbg_dump("G1L", D["G1L"][:])


# --------------------------------------------------------------------------
# Entry point
# --------------------------------------------------------------------------

def _make_in_maps(inputs, shared, percore):
    f32 = np.float32

    def rep(v, shape):
        return np.ascontiguousarray(
            np.broadcast_to(np.asarray(v, f32).reshape(shape),
                            (128,) + tuple(shape[1:])))

    base = {
        "x": np.ascontiguousarray(inputs["x"], f32),
        "E_emb": np.ascontiguousarray(inputs["E_emb"], f32),
        "conv_b": np.ascontiguousarray(inputs["conv_b"].reshape(128, 1), f32),
        "lin2_b": np.ascontiguousarray(inputs["lin2_b"].reshape(64, 1), f32),
        "g1_bl": np.ascontiguousarray(inputs["g1_bl"].reshape(128, 1), f32),
        "g1_br": np.ascontiguousarray(inputs["g1_br"].reshape(128, 1), f32),
        "rec_b": np.ascontiguousarray(inputs["rec_b"].reshape(64, 1), f32),
        "att1_rep": rep(inputs["g1_att"], (1, F)),
        "att2_rep": rep(inputs["g2_att"], (1, F)),
        "g1bias_rep": rep(inputs["g1_bias"], (1, C2)),
        "g2bias_rep": rep(inputs["g2_bias"], (1, C2)),
        "blr_rep": rep(inputs["g2_bl"] + inputs["g2_br"], (1, F)),
    }
    for nm in ("node_proj", "emb_proj", "conv_w0", "conv_w1", "lin2_w",
               "masked_proj", "normal_proj", "g1_wl", "g1_wr", "g2_wl",
               "g2_wr", "rec_w"):
        base[nm] = np.ascontiguousarray(inputs[nm], f32)
    base.update({k: np.ascontiguousarray(v) for k, v in shared.items()})
    in_maps = []
    for c in range(NCORES):
        m = dict(base)
        m.update({k: np.ascontiguousarray(v) for k, v in percore[c].items()})
        in_maps.append(m)
    return in_maps


_CACHE = {}
TRACE = False          # set by test.py to capture NTFF profiles
LRELU_ACT = False      # ACT Lrelu mis-handles alpha on HW; use DVE mul+max
LAST_RESULT = None


def kernel(**inputs):
    global LAST_RESULT
    inputs = {k: np.asarray(v) for k, v in inputs.items()}
    shared, percore, dims = _build_tables(inputs["edge_index"])
    key = (dims["SLOTS"], dims["K2"], LRELU_ACT)
    if key not in _CACHE:
        _CACHE[key] = _build_program(dims, lrelu_act=LRELU_ACT)
    nc = _CACHE[key]
    in_maps = _make_in_maps(inputs, shared, percore)
    kw = {}
    if TRACE:
        kw = dict(trace=True, trace_cores=list(range(NCORES)))
    res = run_bass_kernel_spmd(nc, in_maps, core_ids=list(range(NCORES)),
                               **kw)
    LAST_RESULT = res
    out = np.concatenate([res.results[c]["out"] for c in range(NCORES)],
                         axis=0)
    return out.astype(np.float32)



# revision 10
# speedup vs baseline: 1.8791x; 1.8791x over previous
"""Trainium2 Bass kernel for nn_NodeDetector (masked-node GATv2 ensemble).

v2: all gathers/scatters are tensor-engine one-hot matmuls (fp8 one-hot
lhsT x bf16 hi/lo value tables -> fp32-exact), everything SBUF-resident,
edges processed dense edge-major sorted by dst (34 tiles of 128), per-dst
softmax sums accumulated in PSUM via fp32 one-hot scatter matmuls.

Tricks:
- logit = att.lrelu(u) = 0.6*(a_l[src]+a_r[dst]) + 0.4*att.|u| where
  a_* are per-node scalars appended as cols 128:130 of the value tables
  (so they arrive with the same gather matmul); |u| runs on the Scalar
  engine straight out of PSUM.
- num[d] = sum_e w*u - den[d]*xr[d] (u = xl+xr), so only u is gathered.
  We keep num NEGATED (den*xr - sum w*u) to use the (a op0 s) op1 b DVE.
- w*u scaling runs on the Scalar engine (activation Copy with per-row
  scale AP) straight out of PSUM.
- phase A computes only the <=128 per-core "rare" light pairs (v,d)
  actually consumed by layer 2.

Per core: 32 variants; phases 0/P1 replicated; no collectives.
"""

import numpy as np
import ml_dtypes

import concourse.bass as bass
import concourse.mybir as mybir
import concourse.tile as tile
from concourse import bacc
from concourse.bass_utils import run_bass_kernel_spmd
from concourse.masks import make_identity

F32 = mybir.dt.float32
BF16 = mybir.dt.bfloat16
FP8 = mybir.dt.float8e4
AF = mybir.ActivationFunctionType
OP = mybir.AluOpType
AX = mybir.AxisListType
FP8NP = ml_dtypes.float8_e4m3

N = 256
F = 128
C2 = 64
NH = 2
NCORES = 8
VPC = 32
ET_P1 = 34
W = 130          # value-table width: 128 cols + 2 attention a-cols


# ------------------------------------------------------------------
# host tables
# ------------------------------------------------------------------

def _build_tables(edge_index):
    src = np.asarray(edge_index[0]).astype(np.int64)
    dst = np.asarray(edge_index[1]).astype(np.int64)
    E = src.shape[0]
    order = np.argsort(dst, kind="stable")
    p1_src, p1_dst = src[order], dst[order]

    p1src8 = np.zeros((128, ET_P1 * 2 * 128), FP8NP)
    dst_chunks, sc_halves = [], []
    dst_blocks, sc_blocks = [], []
    for t in range(ET_P1):
        es = slice(128 * t, 128 * (t + 1))
        s_t, d_t = p1_src[es], p1_dst[es]
        for c in range(2):
            m = (s_t // 128) == c
            blk = np.zeros((128, 128), np.float32)
            blk[s_t[m] - 128 * c, np.where(m)[0]] = 1.0
            p1src8[:, (2 * t + c) * 128:(2 * t + c + 1) * 128] = \
                blk.astype(FP8NP)
        dl, sl = [], []
        for c in range(2):
            m = (d_t // 128) == c
            if m.any():
                oh = np.zeros((128, 128), np.float32)
                oh[d_t[m] - 128 * c, np.where(m)[0]] = 1.0
                dl.append(c)
                dst_blocks.append(oh.astype(FP8NP))
                sc = np.zeros((128, 128), np.float32)
                sc[np.where(m)[0], d_t[m] - 128 * c] = 1.0
                sl.append(c)
                sc_blocks.append(sc)
        dst_chunks.append(tuple(dl))
        sc_halves.append(tuple(sl))
    p1dst8 = np.concatenate(dst_blocks, axis=1)
    p1sc32 = np.concatenate(sc_blocks, axis=1).astype(np.float32)

    in_edges_of = [np.where((dst == v) & (src != v))[0] for v in range(N)]
    out_cnt = {}
    for e in range(E):
        if src[e] != dst[e]:
            out_cnt.setdefault(int(src[e]), {})
            d = int(dst[e])
            out_cnt[int(src[e])][d] = out_cnt[int(src[e])].get(d, 0) + 1
    m_self = np.array([((src == v) & (dst == v)).sum() for v in range(N)],
                      np.float32)

    pre = []
    EBs = []
    for c in range(NCORES):
        V = list(range(VPC * c, VPC * (c + 1)))
        el = np.concatenate([in_edges_of[v] for v in V])
        el = el[np.argsort(dst[el], kind="stable")]
        in_set = [set(src[in_edges_of[v]].tolist()) for v in V]
        rare = []
        for vi, v in enumerate(V):
            for d in sorted(out_cnt.get(v, {})):
                if d in in_set[vi]:
                    rare.append((vi, d, out_cnt[v][d]))
        assert len(rare) <= 128, f"rare overflow {len(rare)}"
        EBs.append(-(-len(el) // 128))
        pre.append((V, el, rare))
    EB = max(EBs)

    percore = []
    for c in range(NCORES):
        V, el, rare = pre[c]
        nE = len(el)
        b_src = np.zeros((128, EB * 2 * 128), np.float32)
        d_src = np.zeros((128, EB * 3 * 128), np.float32)
        xr_oh = np.zeros((32, EB * 128), np.float32)
        sc_oh = np.zeros((128, EB * 32), np.float32)
        rare_pos = {(vi, d): i for i, (vi, d, _) in enumerate(rare)}
        for t in range(EB):
            for i in range(128):
                k = 128 * t + i
                if k >= nE:
                    continue
                e = el[k]
                s, v = int(src[e]), int(dst[e])
                vi = v - 32 * c
                ch = s // 128
                b_src[s - 128 * ch, (2 * t + ch) * 128 + i] = 1.0
                if (vi, s) in rare_pos:
                    d_src[rare_pos[(vi, s)], (3 * t + 2) * 128 + i] = 1.0
                else:
                    d_src[s - 128 * ch, (3 * t + ch) * 128 + i] = 1.0
                xr_oh[vi, 128 * t + i] = 1.0
                sc_oh[i, 32 * t + vi] = 1.0
        sv = np.zeros((128, 64), np.float32)
        for vi, v in enumerate(V):
            sv[v % 128, 32 * (v // 128) + vi] = 1.0
        a_d = np.zeros((128, 256), np.float32)
        a_xls = np.zeros((32, 128), np.float32)
        a_xl = np.zeros((128, 256), np.float32)
        a_C = np.zeros((128, 1), np.float32)
        for i, (vi, d, cnt) in enumerate(rare):
            a_d[d % 128, 128 * (d // 128) + i] = 1.0
            a_xls[vi, i] = 1.0
            v = V[vi]
            a_xl[v % 128, 128 * (v // 128) + i] = 1.0
            a_C[i, 0] = cnt
        percore.append({
            "bsrc8": b_src.astype(FP8NP), "dsrc8": d_src.astype(FP8NP),
            "xr8": xr_oh.astype(FP8NP), "bsc32": sc_oh,
            "sv8": sv.astype(FP8NP),
            "selfdiag": np.diag(m_self[V]).astype(np.float32),
            "a_d8": a_d.astype(FP8NP), "a_xls8": a_xls.astype(FP8NP),
            "a_xl8": a_xl.astype(FP8NP), "a_C": a_C,
        })

    shared = {"p1src8": p1src8, "p1dst8": p1dst8, "p1sc32": p1sc32}
    dims = dict(EB=EB, dst_chunks=tuple(dst_chunks),
                sc_halves=tuple(sc_halves), n_dst=p1dst8.shape[1] // 128,
                n_sc=p1sc32.shape[1] // 128)
    return shared, percore, dims


def _prep_weights(inp):
    f32 = np.float32
    w = {k: np.asarray(v, f32) for k, v in inp.items() if k != "edge_index"}
    att1, att2 = w["g1_att"], w["g2_att"]

    def acol(wmat, att):
        return np.stack([wmat[:, 64 * h:64 * (h + 1)] @ att[h]
                         for h in range(NH)], axis=1).astype(f32)

    def rep(v):
        v = np.asarray(v, f32).reshape(1, -1)
        return np.ascontiguousarray(
            np.broadcast_to(v, (128, v.shape[1])))

    blr = w["g2_bl"] + w["g2_br"]
    acb2 = np.stack([blr[64 * h:64 * (h + 1)] @ att2[h] for h in range(NH)])
    P = {
        "w1la_acol": acol(w["g1_wl"], att1),
        "w1ra_acol": acol(w["g1_wr"], att1),
        "acb_l_rep": rep(np.stack([w["g1_bl"][64 * h:64 * (h + 1)] @ att1[h]
                                   for h in range(NH)])),
        "acb_r_rep": rep(np.stack([w["g1_br"][64 * h:64 * (h + 1)] @ att1[h]
                                   for h in range(NH)])),
        "W2LA": np.concatenate([w["g2_wl"], acol(w["g2_wl"], att2)], axis=1),
        "W2RA": np.concatenate([w["g2_wr"], acol(w["g2_wr"], att2)], axis=1),
        "blra_rep": rep(np.concatenate([blr, acb2])),
        "att1_rep04": rep(np.concatenate([att1[0], att1[1]]) * 0.4),
        "att2_rep04": rep(np.concatenate([att2[0], att2[1]]) * 0.4),
        "g1bias_rep": rep(w["g1_bias"]),
        "g2bias_rep": rep(w["g2_bias"]),
        "conv_b": w["conv_b"].reshape(128, 1),
        "lin2_b": w["lin2_b"].reshape(64, 1),
        "g1_bl": w["g1_bl"].reshape(128, 1),
        "g1_br": w["g1_br"].reshape(128, 1),
        "rec_b": w["rec_b"].reshape(64, 1),
    }
    for nm in ("x", "E_emb", "node_proj", "emb_proj", "conv_w0", "conv_w1",
               "lin2_w", "masked_proj", "normal_proj", "g1_wl", "g1_wr",
               "rec_w"):
        P[nm] = w[nm]
    return P


# ------------------------------------------------------------------
# device program
# ------------------------------------------------------------------

def _build_program(dims):
    EB = dims["EB"]
    nc = bacc.Bacc("TRN2", target_bir_lowering=False, debug=False)

    D = {}

    def inp(name, shape, dtype=F32):
        D[name] = nc.dram_tensor(name, list(shape), dtype,
                                 kind="ExternalInput")

    inp("x", [N, 64])
    inp("E_emb", [N, 64])
    for nm, sh in [("node_proj", [64, 128]), ("emb_proj", [64, 128]),
                   ("conv_w0", [128, 128]), ("conv_w1", [128, 128]),
                   ("conv_b", [128, 1]), ("lin2_w", [128, 64]),
                   ("lin2_b", [64, 1]), ("masked_proj", [64, 64]),
                   ("normal_proj", [64, 64]), ("g1_wl", [64, 128]),
                   ("g1_bl", [128, 1]), ("g1_wr", [64, 128]),
                   ("g1_br", [128, 1]), ("rec_w", [64, 64]),
                   ("rec_b", [64, 1]), ("w1la_acol", [64, 2]),
                   ("w1ra_acol", [64, 2]), ("acb_l_rep", [128, 2]),
                   ("acb_r_rep", [128, 2]), ("W2LA", [64, W]),
                   ("W2RA", [64, W]), ("blra_rep", [128, W]),
                   ("att1_rep04", [128, 128]), ("att2_rep04", [128, 128]),
                   ("g1bias_rep", [128, 64]), ("g2bias_rep", [128, 64]),
                   ("selfdiag", [32, 32]), ("a_C", [128, 1]),
                   ("p1sc32", [128, dims["n_sc"] * 128]),
                   ("bsc32", [128, EB * 32])]:
        inp(nm, sh)
    for nm, sh in [("p1src8", [128, ET_P1 * 2 * 128]),
                   ("p1dst8", [128, dims["n_dst"] * 128]),
                   ("bsrc8", [128, EB * 2 * 128]),
                   ("dsrc8", [128, EB * 3 * 128]),
                   ("xr8", [32, EB * 128]), ("sv8", [128, 64]),
                   ("a_d8", [128, 256]), ("a_xls8", [32, 128]),
                   ("a_xl8", [128, 256])]:
        inp(nm, sh, FP8)
    D["out"] = nc.dram_tensor("out", [VPC, 64], F32, kind="ExternalOutput")

    with tile.TileContext(nc) as tc:
        _trace(nc, tc, D, dims)
    nc.compile()
    return nc


def _trace(nc, tc, D, dims):
    import contextlib
    EB = dims["EB"]
    dst_chunks = dims["dst_chunks"]
    sc_halves = dims["sc_halves"]

    ctx = contextlib.ExitStack()
    with ctx:
        consts = ctx.enter_context(tc.tile_pool(name="consts", bufs=1))
        tabs = ctx.enter_context(tc.tile_pool(name="tabs", bufs=1))
        work = ctx.enter_context(tc.tile_pool(name="work", bufs=2))
        psacc = ctx.enter_context(tc.tile_pool(name="psacc", bufs=1,
                                               space="PSUM"))
        psum = ctx.enter_context(tc.tile_pool(name="psum", bufs=2,
                                              space="PSUM"))

        dma = nc.sync.dma_start
        tt = nc.vector.tensor_tensor
        stt = nc.vector.scalar_tensor_tensor
        red = nc.vector.tensor_reduce
        act = nc.scalar.activation
        mm = nc.tensor.matmul

        ident = consts.tile([128, 128], F32, tag="ident")
        make_identity(nc, ident[:])

        def load(name, shape, dtype=F32):
            t_ = consts.tile(list(shape), dtype, tag="c_" + name)
            dma(out=t_[:], in_=D[name][:])
            return t_

        # weights first (phase-0 deps), one-hot packs after
        w_node = load("node_proj", [64, 128])
        w_emb = load("emb_proj", [64, 128])
        w_c0 = load("conv_w0", [128, 128])
        w_c1 = load("conv_w1", [128, 128])
        b_conv = load("conv_b", [128, 1])
        w_lin2 = load("lin2_w", [128, 64])
        b_lin2 = load("lin2_b", [64, 1])
        w_mask = load("masked_proj", [64, 64])
        w_norm = load("normal_proj", [64, 64])
        w_1l = load("g1_wl", [64, 128])
        b_1l = load("g1_bl", [128, 1])
        w_1r = load("g1_wr", [64, 128])
        b_1r = load("g1_br", [128, 1])
        w1la = load("w1la_acol", [64, 2])
        w1ra = load("w1ra_acol", [64, 2])
        acbl = load("acb_l_rep", [128, 2])
        acbr = load("acb_r_rep", [128, 2])
        w2la = load("W2LA", [64, W])
        w2ra = load("W2RA", [64, W])
        blra = load("blra_rep", [128, W])
        att1r = load("att1_rep04", [128, 128])
        att2r = load("att2_rep04", [128, 128])
        g1bias = load("g1bias_rep", [128, 64])
        g2bias = load("g2bias_rep", [128, 64])
        w_rec = load("rec_w", [64, 64])
        b_rec = load("rec_b", [64, 1])
        sdiag = load("selfdiag", [32, 32])
        a_C = load("a_C", [128, 1])

        p1src8 = load("p1src8", [128, ET_P1 * 2 * 128], FP8)
        p1dst8 = load("p1dst8", [128, dims["n_dst"] * 128], FP8)
        p1sc32 = load("p1sc32", [128, dims["n_sc"] * 128])
        bsrc8 = load("bsrc8", [128, EB * 2 * 128], FP8)
        dsrc8 = load("dsrc8", [128, EB * 3 * 128], FP8)
        xr8 = load("xr8", [32, EB * 128], FP8)
        bsc32 = load("bsc32", [128, EB * 32])
        sv8 = load("sv8", [128, 64], FP8)
        a_d8 = load("a_d8", [128, 256], FP8)
        a_xls8 = load("a_xls8", [32, 128], FP8)
        a_xl8 = load("a_xl8", [128, 256], FP8)

        # ---------------- small helpers ----------------
        def ts_mul(out, in0, s):
            nc.vector.tensor_scalar_mul(out=out, in0=in0, scalar1=s)

        def hilo(dst_f32_ap, tag, width):
            """Make bf16 hi/lo pair tiles for a [P, width] f32 ap."""
            P = dst_f32_ap.shape[0]
            hi = tabs.tile([P, width], BF16, tag=tag + "_hi")
            lo32 = work.tile([P, width], F32, tag=tag + "_lo32")
            lo = tabs.tile([P, width], BF16, tag=tag + "_lo")
            nc.vector.tensor_copy(out=hi[:], in_=dst_f32_ap)
            tt(out=lo32[:P, :], in0=dst_f32_ap, in1=hi[:], op=OP.subtract)
            nc.vector.tensor_copy(out=lo[:], in_=lo32[:P, :])
            return hi, lo

        def elu(x_ap, R, tag):
            xp = work.tile([R, 64], F32, tag=tag + "_xp")
            nc.vector.tensor_scalar_max(out=xp[:], in0=x_ap, scalar1=0.0)
            nc.vector.tensor_scalar_min(out=x_ap, in0=x_ap, scalar1=0.0)
            act(out=x_ap, in_=x_ap, func=AF.Exp)
            nc.vector.tensor_scalar_add(out=x_ap, in0=x_ap, scalar1=-1.0)
            tt(out=x_ap, in0=x_ap, in1=xp[:], op=OP.add)

        # ---------------- phase 0 ----------------
        def mm_to_sbuf(lhsT, rhs, M, Nf, tag, bias=None, func=AF.Identity,
                       extra=None):
            out_t = tabs.tile([M, Nf], F32, tag=tag)
            ps = psum.tile([128, 256], F32, tag="ps")
            mm(ps[:M, :Nf], lhsT, rhs, start=True, stop=extra is None)
            if extra is not None:
                mm(ps[:M, :Nf], extra[0], extra[1], start=False, stop=True)
            if bias is None:
                act(out=out_t[:], in_=ps[:M, :Nf], func=func)
            else:
                act(out=out_t[:], in_=ps[:M, :Nf], func=func, bias=bias)
            return out_t

        xT = tabs.tile([64, 256], F32, tag="xT")
        eT = tabs.tile([64, 256], F32, tag="eT")
        for h in range(2):
            for (dname, dstT, tg) in (("x", xT, "ldx"), ("E_emb", eT, "lde")):
                tin = work.tile([128, 64], F32, tag="ph0_" + tg)
                dma(out=tin[:], in_=D[dname][128 * h:128 * (h + 1), :])
                pst = psum.tile([64, 128], F32, tag="ps")
                nc.tensor.transpose(pst[:], tin[:], ident[:])
                nc.vector.tensor_copy(out=dstT[:, 128 * h:128 * (h + 1)],
                                      in_=pst[:])

        xpT = mm_to_sbuf(w_node[:], xT[:], 128, 256, "xpT")
        epT = mm_to_sbuf(w_emb[:], eT[:], 128, 256, "epT")
        HbT = mm_to_sbuf(w_c0[:], epT[:], 128, 256, "HbT", bias=b_conv[:],
                         func=AF.Tanh, extra=(w_c1[:], xpT[:]))
        HsT = mm_to_sbuf(w_c0[:], epT[:], 128, 256, "HsT", bias=b_conv[:],
                         func=AF.Tanh)
        MbT = mm_to_sbuf(w_lin2[:], HbT[:], 64, 256, "MbT", bias=b_lin2[:])
        MsT = mm_to_sbuf(w_lin2[:], HsT[:], 64, 256, "MsT", bias=b_lin2[:])
        PbT = mm_to_sbuf(w_norm[:], MbT[:], 64, 256, "PbT")
        PsT = mm_to_sbuf(w_mask[:], MsT[:], 64, 256, "PsT")

        # value tables VTAB [128, 4*260]: XL | XR | XLs | XRs, each half
        # block [*, ch*130 : ch*130+130] = (vals 128 | acols 2)
        VT = tabs.tile([128, 4 * 2 * W], F32, tag="VT")
        OFF = {"XL": 0, "XR": 2 * W, "XLs": 4 * W, "XRs": 6 * W}

        for (key, PT, wl, bl, aw, acb) in (
                ("XL", PbT, w_1l, b_1l, w1la, acbl),
                ("XR", PbT, w_1r, b_1r, w1ra, acbr),
                ("XLs", PsT, w_1l, b_1l, w1la, acbl),
                ("XRs", PsT, w_1r, b_1r, w1ra, acbr)):
            mainT = mm_to_sbuf(wl[:], PT[:], 128, 256, "mainT_" + key,
                               bias=bl[:])
            for ch in range(2):
                ps = psum.tile([128, 128], F32, tag="ps")
                nc.tensor.transpose(ps[:], mainT[:, 128 * ch:128 * (ch + 1)],
                                    ident[:])
                nc.vector.tensor_copy(
                    out=VT[:, OFF[key] + W * ch:OFF[key] + W * ch + 128],
                    in_=ps[:])
                psa = psum.tile([128, 2], F32, tag="ps")
                mm(psa[:], PT[:, 128 * ch:128 * (ch + 1)], aw[:],
                   start=True, stop=True)
                tt(out=VT[:, OFF[key] + W * ch + 128:OFF[key] + W * ch + W],
                   in0=psa[:], in1=acb[:, 0:2], op=OP.add)

        VThi, VTlo = hilo(VT[:], "VT", 4 * 2 * W)

        def vslice(t_, key, ch, width=W):
            return t_[:, OFF[key] + W * ch:OFF[key] + W * ch + width]

        # ---------------- shared edge-group machinery ----------------
        NG = 3

        def edge_groups(tag, n_et, gather_emit, scatter_emit, att_rep):
            """Process n_et etiles in groups of NG through the edge stage."""
            for g0 in range(0, n_et, NG):
                ng = min(NG, n_et - g0)
                ps_u = psum.tile([128, NG * W], F32, tag="psu")
                for i in range(ng):
                    gather_emit(g0 + i, ps_u[:, W * i:W * (i + 1)])
                psv = ps_u[:].rearrange("p (i c) -> p i c", i=NG)
                absu = work.tile([128, NG, 128], F32, tag=tag + "_absu")
                act(out=absu[:, :ng, :], in_=psv[:, :ng, 0:128], func=AF.Abs)
                tt(out=absu[:, :ng, :], in0=absu[:, :ng, :],
                   in1=att_rep[:].rearrange("p c -> p () c")
                   .to_broadcast([128, ng, 128]), op=OP.mult)
                lgabs = work.tile([128, NG, 2], F32, tag=tag + "_lgabs")
                red(out=lgabs[:, :ng, :],
                    in_=absu[:, :ng, :].rearrange("p i (h f) -> p i h f",
                                                  h=2),
                    axis=AX.X, op=OP.add)
                logit = work.tile([128, NG, 2], F32, tag=tag + "_logit")
                stt(out=logit[:, :ng, :], in0=psv[:, :ng, 128:130],
                    scalar=0.6, in1=lgabs[:, :ng, :], op0=OP.mult,
                    op1=OP.add)
                rhs = work.tile([128, NG, W], F32, tag=tag + "_rhs")
                act(out=rhs[:, :ng, 128:130], in_=logit[:, :ng, :],
                    func=AF.Exp)
                for i in range(ng):
                    for h in range(NH):
                        act(out=rhs[:, i, 64 * h:64 * (h + 1)],
                            in_=psv[:, i, 64 * h:64 * (h + 1)],
                            func=AF.Copy,
                            scale=rhs[:, i, 128 + h:129 + h])
                for i in range(ng):
                    scatter_emit(g0 + i, rhs[:, i, :])

        def gather_seq(sl, pairs):
            """pairs: list of (lhsT_ap, hi_ap, lo_ap); accumulate into sl."""
            n = len(pairs)
            for j, (lhs, hi_ap, lo_ap) in enumerate(pairs):
                mm(sl, lhs, hi_ap, start=(j == 0), stop=False,
                   skip_group_check=True)
                mm(sl, lhs, lo_ap, start=False, stop=(j == n - 1),
                   skip_group_check=True)

        def edge_stage_small(u_sb, R, att_rep, tag):
            """u_sb [R, 130] SBUF -> rhs [R, 130] f32 (w*u | w)."""
            absu = work.tile([R, 128], F32, tag=tag + "_absu")
            act(out=absu[:], in_=u_sb[:R, 0:128], func=AF.Abs)
            tt(out=absu[:], in0=absu[:], in1=att_rep[:R, :], op=OP.mult)
            lgabs = work.tile([R, 2], F32, tag=tag + "_lgabs")
            red(out=lgabs[:], in_=absu[:].rearrange("p (h f) -> p h f", h=2),
                axis=AX.X, op=OP.add)
            logit = work.tile([R, 2], F32, tag=tag + "_logit")
            stt(out=logit[:], in0=u_sb[:R, 128:130], scalar=0.6,
                in1=lgabs[:], op0=OP.mult, op1=OP.add)
            rhs = work.tile([R, W], F32, tag=tag + "_rhs")
            act(out=rhs[:, 128:130], in_=logit[:], func=AF.Exp)
            for h in range(NH):
                act(out=rhs[:, 64 * h:64 * (h + 1)],
                    in_=u_sb[:R, 64 * h:64 * (h + 1)], func=AF.Copy,
                    scale=rhs[:, 128 + h:129 + h])
            return rhs

        def nd_post(ps_acc, xr_sb, bias_rep, R, tag, clamp=False):
            """accumulated (sum w*u | den) + xr -> g = elu(mean+bias)."""
            den = work.tile([R, 2], F32, tag=tag + "_den")
            act(out=den[:], in_=ps_acc[:R, 128:130], func=AF.Copy)
            if clamp:
                nc.vector.tensor_scalar_max(out=den[:], in0=den[:],
                                            scalar1=1e-30)
            nn = work.tile([R, 128], F32, tag=tag + "_nn")
            for hd in range(NH):
                stt(out=nn[:, 64 * hd:64 * (hd + 1)],
                    in0=xr_sb[:R, 64 * hd:64 * (hd + 1)],
                    scalar=den[:, hd:hd + 1], in1=ps_acc[:R, 64 * hd:64 * (hd + 1)],
                    op0=OP.mult, op1=OP.subtract)
            recm = work.tile([R, 2], F32, tag=tag + "_recm")
            nc.vector.reciprocal(out=recm[:], in_=den[:])
            ts_mul(recm[:], recm[:], -0.5)
            g = tabs.tile([R, 64], F32, tag=tag + "_g")
            r1 = work.tile([R, 64], F32, tag=tag + "_r1")
            ts_mul(g[:], nn[:, 0:64], recm[:, 0:1])
            ts_mul(r1[:], nn[:, 64:128], recm[:, 1:2])
            tt(out=g[:], in0=g[:], in1=r1[:], op=OP.add)
            tt(out=g[:], in0=g[:], in1=bias_rep[:R, :], op=OP.add)
            elu(g[:], R, tag + "_elu")
            return g

        # ---------------- minis (XLs/XRs rows of this core) ----------
        def extract_mini(key, tag):
            ps = psum.tile([32, W], F32, tag="ps")
            pairs = [(sv8[:, 32 * ch:32 * (ch + 1)], vslice(VThi, key, ch),
                      vslice(VTlo, key, ch)) for ch in range(2)]
            gather_seq(ps[:], pairs)
            m_ = tabs.tile([32, W], F32, tag=tag)
            nc.vector.tensor_copy(out=m_[:], in_=ps[:])
            hi, lo = hilo(m_[:], tag, W)
            return m_, hi, lo

        XLsm, XLsm_hi, XLsm_lo = extract_mini("XLs", "XLsm")
        XRsm, XRsm_hi, XRsm_lo = extract_mini("XRs", "XRsm")

        # ---------------- B: per-variant layer-1 recompute ------------
        ps_bd = psacc.tile([32, 2 * W], F32, tag="ps_bd")
        ps_b = ps_bd[:, 0:W]
        u_self = tabs.tile([32, W], F32, tag="u_self")
        tt(out=u_self[:], in0=XLsm[:], in1=XRsm[:], op=OP.add)
        rhsS = edge_stage_small(u_self, 32, att1r, "bself")
        mm(ps_b, sdiag[:], rhsS[:], start=True, stop=False,
           skip_group_check=True)

        def b_gather(t, sl):
            pairs = [(bsrc8[:, (2 * t + ch) * 128:(2 * t + ch + 1) * 128],
                      vslice(VThi, "XL", ch), vslice(VTlo, "XL", ch))
                     for ch in range(2)]
            pairs.append((xr8[:, 128 * t:128 * (t + 1)], XRsm_hi[:],
                          XRsm_lo[:]))
            gather_seq(sl, pairs)

        def b_scatter(t, rhs_ap):
            mm(ps_b, bsc32[:, 32 * t:32 * (t + 1)], rhs_ap,
               start=False, stop=(t == EB - 1), skip_group_check=True)

        edge_groups("B", EB, b_gather, b_scatter, att1r)
        g1self = nd_post(ps_b, XRsm[:], g1bias, 32, "bpost")

        # ---------------- P1: base layer 1 ----------------------------
        ps_num = psacc.tile([128, 2 * W], F32, tag="ps_num")
        dst_pos = {}
        pos = 0
        for t in range(ET_P1):
            for c in dst_chunks[t]:
                dst_pos[(t, c)] = pos
                pos += 1
        sc_pos = {}
        pos = 0
        for t in range(ET_P1):
            for h in sc_halves[t]:
                sc_pos[(t, h)] = pos
                pos += 1
        first_h = {h: min(t for t in range(ET_P1) if h in sc_halves[t])
                   for h in range(2)}
        last_h = {h: max(t for t in range(ET_P1) if h in sc_halves[t])
                  for h in range(2)}

        def p1_gather(t, sl):
            pairs = [(p1src8[:, (2 * t + ch) * 128:(2 * t + ch + 1) * 128],
                      vslice(VThi, "XL", ch), vslice(VTlo, "XL", ch))
                     for ch in range(2)]
            for c in dst_chunks[t]:
                j = dst_pos[(t, c)]
                pairs.append((p1dst8[:, 128 * j:128 * (j + 1)],
                              vslice(VThi, "XR", c), vslice(VTlo, "XR", c)))
            gather_seq(sl, pairs)

        def p1_scatter(t, rhs_ap):
            for h in sc_halves[t]:
                j = sc_pos[(t, h)]
                mm(ps_num[:, W * h:W * (h + 1)],
                   p1sc32[:, 128 * j:128 * (j + 1)], rhs_ap,
                   start=(t == first_h[h]), stop=(t == last_h[h]),
                   skip_group_check=True)

        edge_groups("P1", ET_P1, p1_gather, p1_scatter, att1r)

        # T1N [128, 2*130] (negnum | den per half) + g1 base + T2base
        TB = tabs.tile([128, 4 * W], F32, tag="TB")   # T1N halves | T2b halves

        def nd_post_from_t1(TB_, h, bias_rep, tag):
            recm = work.tile([128, 2], F32, tag=tag + "_recm")
            nc.vector.reciprocal(out=recm[:],
                                 in_=TB_[:, W * h + 128:W * h + 130])
            ts_mul(recm[:], recm[:], -0.5)
            g = tabs.tile([128, 64], F32, tag=tag + "_g")
            r1 = work.tile([128, 64], F32, tag=tag + "_r1")
            ts_mul(g[:], TB_[:, W * h:W * h + 64], recm[:, 0:1])
            ts_mul(r1[:], TB_[:, W * h + 64:W * h + 128], recm[:, 1:2])
            tt(out=g[:], in0=g[:], in1=r1[:], op=OP.add)
            tt(out=g[:], in0=g[:], in1=bias_rep[:], op=OP.add)
            elu(g[:], 128, tag + "_elu")
            return g

        g1b_list = []
        for h in range(2):
            pna = ps_num[:, W * h:W * (h + 1)]
            act(out=TB[:, W * h + 128:W * h + 130], in_=pna[:, 128:130],
                func=AF.Copy)
            xroff = OFF["XR"] + W * h
            for hd in range(NH):
                stt(out=TB[:, W * h + 64 * hd:W * h + 64 * (hd + 1)],
                    in0=VT[:, xroff + 64 * hd:xroff + 64 * (hd + 1)],
                    scalar=TB[:, W * h + 128 + hd:W * h + 129 + hd],
                    in1=pna[:, 64 * hd:64 * (hd + 1)],
                    op0=OP.mult, op1=OP.subtract)
            g1b = nd_post_from_t1(TB, h, g1bias, "g1b%d" % h)
            g1b_list.append(g1b)

        # T2base halves into TB cols 2W..4W
        for h in range(2):
            ps = psum.tile([64, 128], F32, tag="ps")
            nc.tensor.transpose(ps[:], g1b_list[h][:, 0:64], ident[:])
            gT = work.tile([64, 128], F32, tag="g1bT")
            nc.vector.tensor_copy(out=gT[:], in_=ps[:])
            ps2 = psum.tile([128, W], F32, tag="ps")
            mm(ps2[:], gT[:], w2la[:], start=True, stop=True)
            nc.vector.tensor_copy(out=TB[:, W * (2 + h):W * (3 + h)],
                                  in_=ps2[:])
        TBhi, TBlo = hilo(TB[:], "TB", 4 * W)

        # ---------------- A: rare light pairs -------------------------
        ps_a1 = psum.tile([128, 2 * W], F32, tag="psA")  # xr_d | t1n_d
        gather_seq(ps_a1[:, 0:W],
                   [(a_d8[:, 128 * ch:128 * (ch + 1)],
                     vslice(VThi, "XR", ch), vslice(VTlo, "XR", ch))
                    for ch in range(2)])
        gather_seq(ps_a1[:, W:2 * W],
                   [(a_d8[:, 128 * ch:128 * (ch + 1)],
                     TBhi[:, W * ch:W * (ch + 1)],
                     TBlo[:, W * ch:W * (ch + 1)]) for ch in range(2)])
        ps_a2 = psum.tile([128, 2 * W], F32, tag="psA")  # xls_v | xl_v
        gather_seq(ps_a2[:, 0:W], [(a_xls8[:], XLsm_hi[:], XLsm_lo[:])])
        gather_seq(ps_a2[:, W:2 * W],
                   [(a_xl8[:, 128 * ch:128 * (ch + 1)],
                     vslice(VThi, "XL", ch), vslice(VTlo, "XL", ch))
                    for ch in range(2)])

        xr_d = work.tile([128, W], F32, tag="a_xrd")
        act(out=xr_d[:], in_=ps_a1[:, 0:W], func=AF.Copy)
        u_n = work.tile([128, W], F32, tag="a_un")
        tt(out=u_n[:], in0=ps_a2[:, 0:W], in1=xr_d[:], op=OP.add)
        u_o = work.tile([128, W], F32, tag="a_uo")
        tt(out=u_o[:], in0=ps_a2[:, W:2 * W], in1=xr_d[:], op=OP.add)

        def a_stage(u_sb, tag):
            absu = work.tile([128, 128], F32, tag=tag + "_absu")
            act(out=absu[:], in_=u_sb[:, 0:128], func=AF.Abs)
            tt(out=absu[:], in0=absu[:], in1=att1r[:], op=OP.mult)
            lgabs = work.tile([128, 2], F32, tag=tag + "_lg")
            red(out=lgabs[:], in_=absu[:].rearrange("p (h f) -> p h f", h=2),
                axis=AX.X, op=OP.add)
            wex = work.tile([128, 2], F32, tag=tag + "_w")
            stt(out=wex[:], in0=u_sb[:, 128:130], scalar=0.6, in1=lgabs[:],
                op0=OP.mult, op1=OP.add)
            act(out=wex[:], in_=wex[:], func=AF.Exp)
            ts_mul(wex[:], wex[:], a_C[:, 0:1])
            return wex

        wn = a_stage(u_n, "a_n")
        wo = a_stage(u_o, "a_o")

        nn_a = work.tile([128, 128], F32, tag="a_nn")
        d_a = work.tile([128, 128], F32, tag="a_d")
        for hd in range(NH):
            cs = slice(64 * hd, 64 * (hd + 1))
            act(out=d_a[:, cs], in_=ps_a2[:, W + 64 * hd:W + 64 * (hd + 1)],
                func=AF.Copy, scale=wo[:, hd:hd + 1])
            act(out=nn_a[:, cs], in_=ps_a2[:, 64 * hd:64 * (hd + 1)],
                func=AF.Copy, scale=wn[:, hd:hd + 1])
        tt(out=d_a[:], in0=d_a[:], in1=nn_a[:], op=OP.subtract)
        for hd in range(NH):
            cs = slice(64 * hd, 64 * (hd + 1))
            tt(out=nn_a[:, cs], in0=d_a[:, cs],
               in1=ps_a1[:, W + 64 * hd:W + 64 * (hd + 1)], op=OP.add)
        den_a = work.tile([128, 2], F32, tag="a_den")
        tt(out=den_a[:], in0=wn[:], in1=wo[:], op=OP.subtract)
        tt(out=den_a[:], in0=den_a[:], in1=ps_a1[:, W + 128:W + 130],
           op=OP.add)
        nc.vector.tensor_scalar_max(out=den_a[:], in0=den_a[:],
                                    scalar1=1e-30)
        recm = work.tile([128, 2], F32, tag="a_recm")
        nc.vector.reciprocal(out=recm[:], in_=den_a[:])
        ts_mul(recm[:], recm[:], -0.5)
        g1light = tabs.tile([128, 64], F32, tag="g1light")
        r1 = work.tile([128, 64], F32, tag="a_r1")
        ts_mul(g1light[:], nn_a[:, 0:64], recm[:, 0:1])
        ts_mul(r1[:], nn_a[:, 64:128], recm[:, 1:2])
        tt(out=g1light[:], in0=g1light[:], in1=r1[:], op=OP.add)
        tt(out=g1light[:], in0=g1light[:], in1=g1bias[:], op=OP.add)
        elu(g1light[:], 128, "a_elu")

        # T2rare
        ps = psum.tile([64, 128], F32, tag="ps")
        nc.tensor.transpose(ps[:], g1light[:], ident[:])
        gT = work.tile([64, 128], F32, tag="g1lT")
        nc.vector.tensor_copy(out=gT[:], in_=ps[:])
        ps2 = psum.tile([128, W], F32, tag="ps")
        mm(ps2[:], gT[:], w2la[:], start=True, stop=True)
        T2r = tabs.tile([128, W], F32, tag="T2r")
        nc.vector.tensor_copy(out=T2r[:], in_=ps2[:])
        T2r_hi, T2r_lo = hilo(T2r[:], "T2r", W)

        # ---------------- XR2S / T2self -------------------------------
        ps = psum.tile([64, 32], F32, tag="ps")
        nc.tensor.transpose(ps[:], g1self[:], ident[:32, :32])
        g1sT = work.tile([64, 32], F32, tag="g1sT")
        nc.vector.tensor_copy(out=g1sT[:], in_=ps[:])
        ps2 = psum.tile([32, W], F32, tag="ps")
        mm(ps2[:], g1sT[:], w2ra[:], start=True, stop=True)
        XR2S = tabs.tile([32, W], F32, tag="XR2S")
        tt(out=XR2S[:], in0=ps2[:], in1=blra[:32, :], op=OP.add)
        XR2S_hi, XR2S_lo = hilo(XR2S[:], "XR2S", W)
        ps3 = psum.tile([32, W], F32, tag="ps")
        mm(ps3[:], g1sT[:], w2la[:], start=True, stop=True)
        u_ds = tabs.tile([32, W], F32, tag="u_ds")
        tt(out=u_ds[:], in0=ps3[:], in1=XR2S[:], op=OP.add)

        # ---------------- D: layer 2 at dst v -------------------------
        ps_d = ps_bd[:, W:2 * W]
        rhsS2 = edge_stage_small(u_ds, 32, att2r, "dself")
        mm(ps_d, sdiag[:], rhsS2[:], start=True, stop=False,
           skip_group_check=True)

        def d_gather(t, sl):
            pairs = [(dsrc8[:, (3 * t + ch) * 128:(3 * t + ch + 1) * 128],
                      TBhi[:, W * (2 + ch):W * (3 + ch)],
                      TBlo[:, W * (2 + ch):W * (3 + ch)]) for ch in range(2)]
            pairs.append((dsrc8[:, (3 * t + 2) * 128:(3 * t + 3) * 128],
                          T2r_hi[:], T2r_lo[:]))
            pairs.append((xr8[:, 128 * t:128 * (t + 1)], XR2S_hi[:],
                          XR2S_lo[:]))
            gather_seq(sl, pairs)

        def d_scatter(t, rhs_ap):
            mm(ps_d, bsc32[:, 32 * t:32 * (t + 1)], rhs_ap,
               start=False, stop=(t == EB - 1), skip_group_check=True)

        edge_groups("DD", EB, d_gather, d_scatter, att2r)
        g2 = nd_post(ps_d, XR2S[:], g2bias, 32, "dpost")

        # ---------------- out -----------------------------------------
        ps = psum.tile([64, 32], F32, tag="ps")
        nc.tensor.transpose(ps[:], g2[:], ident[:32, :32])
        g2T = work.tile([64, 32], F32, tag="g2T")
        nc.vector.tensor_copy(out=g2T[:], in_=ps[:])
        pso = psum.tile([64, 32], F32, tag="ps")
        mm(pso[:], w_rec[:], g2T[:], start=True, stop=True)
        outT = work.tile([64, 32], F32, tag="outT")
        act(out=outT[:], in_=pso[:], func=AF.Tanh, bias=b_rec[:])
        psf = psum.tile([32, 64], F32, tag="ps")
        nc.tensor.transpose(psf[:], outT[:], ident[:64, :64])
        outsb = work.tile([32, 64], F32, tag="outsb")
        nc.vector.tensor_copy(out=outsb[:], in_=psf[:])
        dma(out=D["out"][:], in_=outsb[:])


# ------------------------------------------------------------------
# entry point
# ------------------------------------------------------------------

_CACHE = {}
TRACE = False
LAST_RESULT = None


def kernel(**inputs):
    global LAST_RESULT
    inputs = {k: np.asarray(v) for k, v in inputs.items()}
    shared, percore, dims = _build_tables(inputs["edge_index"])
    P = _prep_weights(inputs)
    key = (dims["EB"], dims["dst_chunks"], dims["sc_halves"])
    if key not in _CACHE:
        _CACHE[key] = _build_program(dims)
    nc = _CACHE[key]
    base = dict(P)
    base.update(shared)
    in_maps = []
    for c in range(NCORES):
        m = dict(base)
        m.update(percore[c])
        in_maps.append({k: np.ascontiguousarray(v) for k, v in m.items()})
    kw = {}
    if TRACE:
        kw = dict(trace=True, trace_cores=list(range(NCORES)))
    res = run_bass_kernel_spmd(nc, in_maps, core_ids=list(range(NCORES)),
                               **kw)
    LAST_RESULT = res
    out = np.concatenate([res.results[c]["out"] for c in range(NCORES)],
                         axis=0)
    return out.astype(np.float32)


# revision 14
# speedup vs baseline: 2.8580x; 1.5209x over previous
"""Trainium2 Bass kernel for nn_NodeDetector (masked-node GATv2 ensemble).

v3: all gathers/scatters are tensor-engine one-hot matmuls (fp8 one-hot
lhsT x bf16 hi/lo value tables -> fp32-exact), everything SBUF-resident,
edges processed dense edge-major sorted by dst (34 tiles of 128), per-dst
softmax sums accumulated in PSUM via fp32 one-hot scatter matmuls.

Tricks:
- logit = att.lrelu(u) = 0.6*(a_l[src]+a_r[dst]) + 0.4*att.|u| where
  a_* are per-node scalars appended as cols 128:130 of the value tables
  (gathered by the same matmul); |u| runs on the Scalar engine out of
  PSUM.
- num[d] = sum_e w*u - den[d]*xr[d] (u = xl+xr), so only u is gathered;
  num is kept NEGATED (den*xr - sum w*u) to fit the fused DVE op.
- phase A computes only the <=128 per-core "rare" light pairs (v,d)
  actually consumed by layer 2.
- emission order overlaps the input DMA of one-hot tables with phase-0
  compute, and D's T2base gathers with phase A's serial chain.

Per core: 32 variants; phases 0/P1 replicated; no collectives.
"""

import numpy as np
import ml_dtypes

import concourse.bass as bass
import concourse.mybir as mybir
import concourse.tile as tile
from concourse import bacc
from concourse.bass_utils import run_bass_kernel_spmd
from concourse.masks import make_identity

F32 = mybir.dt.float32
BF16 = mybir.dt.bfloat16
FP8 = mybir.dt.float8e4
AF = mybir.ActivationFunctionType
OP = mybir.AluOpType
AX = mybir.AxisListType
FP8NP = ml_dtypes.float8_e4m3

N = 256
NH = 2
NCORES = 8
VPC = 32
ET_P1 = 34
W = 130          # value-table width: 128 cols + 2 attention a-cols
NG = 3           # etiles per DVE group (3*130*4B fits one PSUM bank)


# ------------------------------------------------------------------
# host tables
# ------------------------------------------------------------------

def _build_tables(edge_index):
    src = np.asarray(edge_index[0]).astype(np.int64)
    dst = np.asarray(edge_index[1]).astype(np.int64)
    E = src.shape[0]
    order = np.argsort(dst, kind="stable")
    p1_src, p1_dst = src[order], dst[order]

    p1src8 = np.zeros((128, ET_P1 * 2 * 128), FP8NP)
    dst_chunks, sc_halves = [], []
    dst_blocks, sc_blocks = [], []
    for t in range(ET_P1):
        es = slice(128 * t, 128 * (t + 1))
        s_t, d_t = p1_src[es], p1_dst[es]
        for c in range(2):
            m = (s_t // 128) == c
            blk = np.zeros((128, 128), np.float32)
            blk[s_t[m] - 128 * c, np.where(m)[0]] = 1.0
            p1src8[:, (2 * t + c) * 128:(2 * t + c + 1) * 128] = \
                blk.astype(FP8NP)
        dl, sl = [], []
        for c in range(2):
            m = (d_t // 128) == c
            if m.any():
                oh = np.zeros((128, 128), np.float32)
                oh[d_t[m] - 128 * c, np.where(m)[0]] = 1.0
                dl.append(c)
                dst_blocks.append(oh.astype(FP8NP))
                sc = np.zeros((128, 128), np.float32)
                sc[np.where(m)[0], d_t[m] - 128 * c] = 1.0
                sl.append(c)
                sc_blocks.append(sc)
        dst_chunks.append(tuple(dl))
        sc_halves.append(tuple(sl))
    p1dst8 = np.concatenate(dst_blocks, axis=1)
    p1sc32 = np.concatenate(sc_blocks, axis=1).astype(np.float32)

    in_edges_of = [np.where((dst == v) & (src != v))[0] for v in range(N)]
    out_cnt = {}
    for e in range(E):
        if src[e] != dst[e]:
            out_cnt.setdefault(int(src[e]), {})
            d = int(dst[e])
            out_cnt[int(src[e])][d] = out_cnt[int(src[e])].get(d, 0) + 1
    m_self = np.array([((src == v) & (dst == v)).sum() for v in range(N)],
                      np.float32)

    pre = []
    EBs = []
    for c in range(NCORES):
        V = list(range(VPC * c, VPC * (c + 1)))
        el = np.concatenate([in_edges_of[v] for v in V])
        el = el[np.argsort(dst[el], kind="stable")]
        in_set = [set(src[in_edges_of[v]].tolist()) for v in V]
        rare = []
        for vi, v in enumerate(V):
            for d in sorted(out_cnt.get(v, {})):
                if d in in_set[vi]:
                    rare.append((vi, d, out_cnt[v][d]))
        assert len(rare) <= 128, f"rare overflow {len(rare)}"
        EBs.append(-(-len(el) // 128))
        pre.append((V, el, rare))
    EB = max(EBs)

    percore = []
    for c in range(NCORES):
        V, el, rare = pre[c]
        nE = len(el)
        b_src = np.zeros((128, EB * 2 * 128), np.float32)
        d_src = np.zeros((128, EB * 3 * 128), np.float32)
        xr_oh = np.zeros((32, EB * 128), np.float32)
        sc_oh = np.zeros((128, EB * 32), np.float32)
        rare_pos = {(vi, d): i for i, (vi, d, _) in enumerate(rare)}
        for t in range(EB):
            for i in range(128):
                k = 128 * t + i
                if k >= nE:
                    continue
                e = el[k]
                s, v = int(src[e]), int(dst[e])
                vi = v - 32 * c
                ch = s // 128
                b_src[s - 128 * ch, (2 * t + ch) * 128 + i] = 1.0
                if (vi, s) in rare_pos:
                    d_src[rare_pos[(vi, s)], (3 * t + 2) * 128 + i] = 1.0
                else:
                    d_src[s - 128 * ch, (3 * t + ch) * 128 + i] = 1.0
                xr_oh[vi, 128 * t + i] = 1.0
                sc_oh[i, 32 * t + vi] = 1.0
        sv = np.zeros((128, 64), np.float32)
        for vi, v in enumerate(V):
            sv[v % 128, 32 * (v // 128) + vi] = 1.0
        a_d = np.zeros((128, 256), np.float32)
        a_xls = np.zeros((32, 128), np.float32)
        a_xl = np.zeros((128, 256), np.float32)
        a_C = np.zeros((128, 1), np.float32)
        for i, (vi, d, cnt) in enumerate(rare):
            a_d[d % 128, 128 * (d // 128) + i] = 1.0
            a_xls[vi, i] = 1.0
            v = V[vi]
            a_xl[v % 128, 128 * (v // 128) + i] = 1.0
            a_C[i, 0] = cnt
        percore.append({
            "bsrc8": b_src.astype(FP8NP), "dsrc8": d_src.astype(FP8NP),
            "xr8": xr_oh.astype(FP8NP), "bsc32": sc_oh,
            "sv8": sv.astype(FP8NP),
            "selfdiag": np.diag(m_self[V]).astype(np.float32),
            "a_d8": a_d.astype(FP8NP), "a_xls8": a_xls.astype(FP8NP),
            "a_xl8": a_xl.astype(FP8NP), "a_C": a_C,
        })

    shared = {"p1src8": p1src8, "p1dst8": p1dst8, "p1sc32": p1sc32}
    dims = dict(EB=EB, dst_chunks=tuple(dst_chunks),
                sc_halves=tuple(sc_halves), n_dst=p1dst8.shape[1] // 128,
                n_sc=p1sc32.shape[1] // 128)
    return shared, percore, dims


def _prep_weights(inp):
    f32 = np.float32
    w = {k: np.asarray(v, f32) for k, v in inp.items() if k != "edge_index"}
    att1, att2 = w["g1_att"], w["g2_att"]

    def acol(wmat, att):
        return np.stack([wmat[:, 64 * h:64 * (h + 1)] @ att[h]
                         for h in range(NH)], axis=1).astype(f32)

    def rep(v):
        v = np.asarray(v, f32).reshape(1, -1)
        return np.ascontiguousarray(np.broadcast_to(v, (128, v.shape[1])))

    blr = w["g2_bl"] + w["g2_br"]
    acb2 = np.stack([blr[64 * h:64 * (h + 1)] @ att2[h] for h in range(NH)])
    acb_l = np.stack([w["g1_bl"][64 * h:64 * (h + 1)] @ att1[h]
                      for h in range(NH)])
    acb_r = np.stack([w["g1_br"][64 * h:64 * (h + 1)] @ att1[h]
                      for h in range(NH)])
    P = {
        "w1lra_acol": np.concatenate([acol(w["g1_wl"], att1),
                                      acol(w["g1_wr"], att1)], axis=1),
        "acb_lr_rep": rep(np.concatenate([acb_l, acb_r])),
        "W2LA": np.concatenate([w["g2_wl"], acol(w["g2_wl"], att2)], axis=1),
        "W2RA": np.concatenate([w["g2_wr"], acol(w["g2_wr"], att2)], axis=1),
        "blra_rep": rep(np.concatenate([blr, acb2])),
        "att1_rep04": rep(np.concatenate([att1[0], att1[1]]) * 0.4),
        "att2_rep04": rep(np.concatenate([att2[0], att2[1]]) * 0.4),
        "g1bias_rep": rep(w["g1_bias"]),
        "g2bias_rep": rep(w["g2_bias"]),
        "conv_b": w["conv_b"].reshape(128, 1),
        "lin2_b": w["lin2_b"].reshape(64, 1),
        "g1_bl": w["g1_bl"].reshape(128, 1),
        "g1_br": w["g1_br"].reshape(128, 1),
        "rec_b": w["rec_b"].reshape(64, 1),
    }
    for nm in ("x", "E_emb", "node_proj", "emb_proj", "conv_w0", "conv_w1",
               "lin2_w", "masked_proj", "normal_proj", "g1_wl", "g1_wr",
               "rec_w"):
        P[nm] = w[nm]
    return P


# ------------------------------------------------------------------
# device program
# ------------------------------------------------------------------

def _build_program(dims, dbg=False):
    EB = dims["EB"]
    nc = bacc.Bacc("TRN2", target_bir_lowering=False, debug=False)
    D = {"_dbg": dbg, "_nc": nc}

    def inp(name, shape, dtype=F32):
        D[name] = nc.dram_tensor(name, list(shape), dtype,
                                 kind="ExternalInput")

    inp("x", [N, 64])
    inp("E_emb", [N, 64])
    for nm, sh in [("node_proj", [64, 128]), ("emb_proj", [64, 128]),
                   ("conv_w0", [128, 128]), ("conv_w1", [128, 128]),
                   ("conv_b", [128, 1]), ("lin2_w", [128, 64]),
                   ("lin2_b", [64, 1]), ("masked_proj", [64, 64]),
                   ("normal_proj", [64, 64]), ("g1_wl", [64, 128]),
                   ("g1_bl", [128, 1]), ("g1_wr", [64, 128]),
                   ("g1_br", [128, 1]), ("rec_w", [64, 64]),
                   ("rec_b", [64, 1]), ("w1lra_acol", [64, 4]),
                   ("acb_lr_rep", [128, 4]), ("W2LA", [64, W]),
                   ("W2RA", [64, W]), ("blra_rep", [128, W]),
                   ("att1_rep04", [128, 128]), ("att2_rep04", [128, 128]),
                   ("g1bias_rep", [128, 64]), ("g2bias_rep", [128, 64]),
                   ("selfdiag", [32, 32]), ("a_C", [128, 1]),
                   ("p1sc32", [128, dims["n_sc"] * 128]),
                   ("bsc32", [128, EB * 32])]:
        inp(nm, sh)
    for nm, sh in [("p1src8", [128, ET_P1 * 2 * 128]),
                   ("p1dst8", [128, dims["n_dst"] * 128]),
                   ("bsrc8", [128, EB * 2 * 128]),
                   ("dsrc8", [128, EB * 3 * 128]),
                   ("xr8", [32, EB * 128]), ("sv8", [128, 64]),
                   ("a_d8", [128, 256]), ("a_xls8", [32, 128]),
                   ("a_xl8", [128, 256])]:
        inp(nm, sh, FP8)
    D["outT"] = nc.dram_tensor("outT", [64, VPC], F32, kind="ExternalOutput")

    with tile.TileContext(nc) as tc:
        _trace(nc, tc, D, dims)
    nc.compile()
    return nc


def _trace(nc, tc, D, dims):
    import contextlib
    EB = dims["EB"]
    dst_chunks = dims["dst_chunks"]
    sc_halves = dims["sc_halves"]

    ctx = contextlib.ExitStack()
    with ctx:
        consts = ctx.enter_context(tc.tile_pool(name="consts", bufs=1))
        tabs = ctx.enter_context(tc.tile_pool(name="tabs", bufs=1))
        work = ctx.enter_context(tc.tile_pool(name="work", bufs=2))
        psacc = ctx.enter_context(tc.tile_pool(name="psacc", bufs=1,
                                               space="PSUM"))
        psum = ctx.enter_context(tc.tile_pool(name="psum", bufs=2,
                                              space="PSUM"))

        dma = nc.sync.dma_start
        tt = nc.vector.tensor_tensor
        stt = nc.vector.scalar_tensor_tensor
        red = nc.vector.tensor_reduce
        act = nc.scalar.activation
        mm = nc.tensor.matmul

        def dbg_dump(name, ap):
            if not D.get("_dbg"):
                return
            sh = list(ap.shape)
            t_ = nc.dram_tensor("dbg_" + name, sh, F32,
                                kind="ExternalOutput")
            dma(out=t_[:], in_=ap)

        ident = consts.tile([128, 128], F32, tag="ident")
        make_identity(nc, ident[:])

        def load(name, shape, dtype=F32):
            t_ = consts.tile(list(shape), dtype, tag="c_" + name)
            dma(out=t_[:], in_=D[name][:])
            return t_

        # ---- phase-0 inputs FIRST on the DMA queue ----
        xin = [work.tile([128, 64], F32, tag="ph0_x%d" % h,
                         name="xin%d" % h) for h in range(2)]
        ein = [work.tile([128, 64], F32, tag="ph0_e%d" % h,
                         name="ein%d" % h) for h in range(2)]
        for h in range(2):
            dma(out=xin[h][:], in_=D["x"][128 * h:128 * (h + 1), :])
            dma(out=ein[h][:], in_=D["E_emb"][128 * h:128 * (h + 1), :])
        w_node = load("node_proj", [64, 128])
        w_emb = load("emb_proj", [64, 128])
        w_c0 = load("conv_w0", [128, 128])
        w_c1 = load("conv_w1", [128, 128])
        b_conv = load("conv_b", [128, 1])
        w_lin2 = load("lin2_w", [128, 64])
        b_lin2 = load("lin2_b", [64, 1])
        w_mask = load("masked_proj", [64, 64])
        w_norm = load("normal_proj", [64, 64])
        w_1l = load("g1_wl", [64, 128])
        b_1l = load("g1_bl", [128, 1])
        w_1r = load("g1_wr", [64, 128])
        b_1r = load("g1_br", [128, 1])
        w1lra = load("w1lra_acol", [64, 4])
        acblr = load("acb_lr_rep", [128, 4])
        w2la = load("W2LA", [64, W])
        w2ra = load("W2RA", [64, W])
        blra = load("blra_rep", [128, W])
        att1r = load("att1_rep04", [128, 128])
        att2r = load("att2_rep04", [128, 128])
        g1bias = load("g1bias_rep", [128, 64])
        g2bias = load("g2bias_rep", [128, 64])
        w_rec = load("rec_w", [64, 64])
        b_rec = load("rec_b", [64, 1])
        sdiag = load("selfdiag", [32, 32])
        a_C = load("a_C", [128, 1])

        # ---------------- helpers ----------------
        def ts_mul(out, in0, s):
            nc.vector.tensor_scalar_mul(out=out, in0=in0, scalar1=s)

        def hilo(dst_f32_ap, tag, width):
            P = dst_f32_ap.shape[0]
            hi = tabs.tile([P, width], BF16, tag=tag + "_hi")
            lo32 = work.tile([P, width], F32, tag=tag + "_lo32")
            lo = tabs.tile([P, width], BF16, tag=tag + "_lo")
            nc.vector.tensor_copy(out=hi[:], in_=dst_f32_ap)
            tt(out=lo32[:P, :], in0=dst_f32_ap, in1=hi[:], op=OP.subtract)
            nc.vector.tensor_copy(out=lo[:], in_=lo32[:P, :])
            return hi, lo

        def elu(x_ap, R, tag):
            xp = work.tile([R, 64], F32, tag=tag + "_xp")
            nc.vector.tensor_scalar_max(out=xp[:], in0=x_ap, scalar1=0.0)
            nc.vector.tensor_scalar_min(out=x_ap, in0=x_ap, scalar1=0.0)
            act(out=x_ap, in_=x_ap, func=AF.Exp)
            nc.vector.tensor_scalar_add(out=x_ap, in0=x_ap, scalar1=-1.0)
            tt(out=x_ap, in0=x_ap, in1=xp[:], op=OP.add)

        def mm_to_sbuf(lhsT, rhs, M, Nf, tag, bias=None, func=AF.Identity,
                       extra=None):
            out_t = tabs.tile([M, Nf], F32, tag=tag)
            ps = psum.tile([128, 256], F32, tag="ps")
            mm(ps[:M, :Nf], lhsT, rhs, start=True, stop=extra is None)
            if extra is not None:
                mm(ps[:M, :Nf], extra[0], extra[1], start=False, stop=True)
            if bias is None:
                act(out=out_t[:], in_=ps[:M, :Nf], func=func)
            else:
                act(out=out_t[:], in_=ps[:M, :Nf], func=func, bias=bias)
            return out_t

        # ---------------- phase 0 ----------------
        xT = tabs.tile([64, 256], F32, tag="xT")
        eT = tabs.tile([64, 256], F32, tag="eT")
        for h in range(2):
            for (tin, dstT) in ((xin[h], xT), (ein[h], eT)):
                pst = psum.tile([64, 128], F32, tag="ps")
                nc.tensor.transpose(pst[:], tin[:], ident[:])
                nc.vector.tensor_copy(out=dstT[:, 128 * h:128 * (h + 1)],
                                      in_=pst[:])

        xpT = mm_to_sbuf(w_node[:], xT[:], 128, 256, "xpT")
        epT = mm_to_sbuf(w_emb[:], eT[:], 128, 256, "epT")
        HbT = mm_to_sbuf(w_c0[:], epT[:], 128, 256, "HbT", bias=b_conv[:],
                         func=AF.Tanh, extra=(w_c1[:], xpT[:]))
        HsT = mm_to_sbuf(w_c0[:], epT[:], 128, 256, "HsT", bias=b_conv[:],
                         func=AF.Tanh)
        MbT = mm_to_sbuf(w_lin2[:], HbT[:], 64, 256, "MbT", bias=b_lin2[:])
        MsT = mm_to_sbuf(w_lin2[:], HsT[:], 64, 256, "MsT", bias=b_lin2[:])
        PbT = mm_to_sbuf(w_norm[:], MbT[:], 64, 256, "PbT")
        PsT = mm_to_sbuf(w_mask[:], MsT[:], 64, 256, "PsT")

        # value tables VT [128, 8*130]: XL | XR | XLs | XRs (2 chunks each)
        VT = tabs.tile([128, 8 * W], F32, tag="VT")
        OFF = {"XL": 0, "XR": 2 * W, "XLs": 4 * W, "XRs": 6 * W}

        for (kl, kr, PT) in (("XL", "XR", PbT), ("XLs", "XRs", PsT)):
            mainL = mm_to_sbuf(w_1l[:], PT[:], 128, 256, "mainT_" + kl,
                               bias=b_1l[:])
            mainR = mm_to_sbuf(w_1r[:], PT[:], 128, 256, "mainT_" + kr,
                               bias=b_1r[:])
            for ch in range(2):
                for key, mainT in ((kl, mainL), (kr, mainR)):
                    ps = psum.tile([128, 128], F32, tag="ps")
                    nc.tensor.transpose(ps[:],
                                        mainT[:, 128 * ch:128 * (ch + 1)],
                                        ident[:])
                    nc.vector.tensor_copy(
                        out=VT[:, OFF[key] + W * ch:OFF[key] + W * ch + 128],
                        in_=ps[:])
                psa = psum.tile([128, 4], F32, tag="ps")
                mm(psa[:], PT[:, 128 * ch:128 * (ch + 1)], w1lra[:],
                   start=True, stop=True)
                acsb = work.tile([128, 4], F32, tag="acsb")
                tt(out=acsb[:], in0=psa[:], in1=acblr[:, 0:4], op=OP.add)
                nc.vector.tensor_copy(
                    out=VT[:, OFF[kl] + W * ch + 128:OFF[kl] + W * ch + W],
                    in_=acsb[:, 0:2])
                nc.vector.tensor_copy(
                    out=VT[:, OFF[kr] + W * ch + 128:OFF[kr] + W * ch + W],
                    in_=acsb[:, 2:4])

        dbg_dump("VT", VT[:])
        VThi, VTlo = hilo(VT[:], "VT", 8 * W)

        def vslice(t_, key, ch):
            return t_[:, OFF[key] + W * ch:OFF[key] + W * ch + W]

        # ---- one-hot tables (DMA overlaps phase-0 compute) ----
        sv8 = load("sv8", [128, 64], FP8)
        xr8 = load("xr8", [32, EB * 128], FP8)
        bsrc8 = load("bsrc8", [128, EB * 2 * 128], FP8)
        bsc32 = load("bsc32", [128, EB * 32])
        p1src8 = load("p1src8", [128, ET_P1 * 2 * 128], FP8)
        p1dst8 = load("p1dst8", [128, dims["n_dst"] * 128], FP8)
        p1sc32 = load("p1sc32", [128, dims["n_sc"] * 128])
        dsrc8 = load("dsrc8", [128, EB * 3 * 128], FP8)
        a_d8 = load("a_d8", [128, 256], FP8)
        a_xls8 = load("a_xls8", [32, 128], FP8)
        a_xl8 = load("a_xl8", [128, 256], FP8)

        # ---------------- edge machinery ----------------
        def gather_seq(sl, pairs, start=True, stop=True):
            n = len(pairs)
            for j, (lhs, hi_ap, lo_ap) in enumerate(pairs):
                mm(sl, lhs, hi_ap, start=(j == 0 and start), stop=False,
                   skip_group_check=True)
                mm(sl, lhs, lo_ap, start=False,
                   stop=(j == n - 1 and stop), skip_group_check=True)

        def group_stage(ps_u, ng, tag, att_rep):
            """Edge stage for a group of ng etiles in ps_u -> rhs tile."""
            psv = ps_u[:].rearrange("p (i c) -> p i c", i=NG)
            absu = work.tile([128, NG, 128], F32, tag=tag + "_absu")
            act(out=absu[:, :ng, :], in_=psv[:, :ng, 0:128], func=AF.Abs)
            tt(out=absu[:, :ng, :], in0=absu[:, :ng, :],
               in1=att_rep[:].rearrange("p c -> p () c")
               .to_broadcast([128, ng, 128]), op=OP.mult)
            lgabs = work.tile([128, NG, 2], F32, tag=tag + "_lgabs")
            red(out=lgabs[:, :ng, :],
                in_=absu[:, :ng, :].rearrange("p i (h f) -> p i h f", h=2),
                axis=AX.X, op=OP.add)
            logit = work.tile([128, NG, 2], F32, tag=tag + "_logit")
            stt(out=logit[:, :ng, :], in0=psv[:, :ng, 128:130],
                scalar=0.6, in1=lgabs[:, :ng, :], op0=OP.mult, op1=OP.add)
            wexp = work.tile([128, NG, 2], F32, tag=tag + "_wexp")
            act(out=wexp[:, :ng, :], in_=logit[:, :ng, :], func=AF.Exp)
            rhs = work.tile([128, NG, W], F32, tag=tag + "_rhs")
            for h in range(NH):
                tt(out=rhs[:, :ng, 64 * h:64 * (h + 1)],
                   in0=psv[:, :ng, 64 * h:64 * (h + 1)],
                   in1=wexp[:, :ng, h:h + 1].to_broadcast([128, ng, 64]),
                   op=OP.mult)
            nc.vector.tensor_copy(out=rhs[:, :ng, 128:130],
                                  in_=wexp[:, :ng, :])
            return rhs

        def run_groups(tag, n_et, gather_emit, scatter_emit, att_rep):
            for g0 in range(0, n_et, NG):
                ng = min(NG, n_et - g0)
                ps_u = psum.tile([128, NG * W], F32, tag="psu")
                for i in range(ng):
                    gather_emit(g0 + i, ps_u[:, W * i:W * (i + 1)])
                rhs = group_stage(ps_u, ng, tag, att_rep)
                for i in range(ng):
                    scatter_emit(g0 + i, rhs[:, i, :])

        def edge_stage_small(u_sb, R, att_rep, tag):
            absu = work.tile([R, 128], F32, tag=tag + "_absu")
            act(out=absu[:], in_=u_sb[:R, 0:128], func=AF.Abs)
            tt(out=absu[:], in0=absu[:], in1=att_rep[:R, :], op=OP.mult)
            lgabs = work.tile([R, 2], F32, tag=tag + "_lgabs")
            red(out=lgabs[:], in_=absu[:].rearrange("p (h f) -> p h f", h=2),
                axis=AX.X, op=OP.add)
            wexp = work.tile([R, 2], F32, tag=tag + "_wexp")
            stt(out=wexp[:], in0=u_sb[:R, 128:130], scalar=0.6,
                in1=lgabs[:], op0=OP.mult, op1=OP.add)
            act(out=wexp[:], in_=wexp[:], func=AF.Exp)
            rhs = work.tile([R, W], F32, tag=tag + "_rhs")
            for h in range(NH):
                tt(out=rhs[:, 64 * h:64 * (h + 1)],
                   in0=u_sb[:R, 64 * h:64 * (h + 1)],
                   in1=wexp[:, h:h + 1].to_broadcast([R, 64]), op=OP.mult)
            nc.vector.tensor_copy(out=rhs[:, 128:130], in_=wexp[:])
            return rhs

        def nd_post(ps_acc, xr_sb, bias_rep, R, tag):
            den = work.tile([R, 2], F32, tag=tag + "_den")
            act(out=den[:], in_=ps_acc[:R, 128:130], func=AF.Copy)
            nn = work.tile([R, 128], F32, tag=tag + "_nn")
            for hd in range(NH):
                stt(out=nn[:, 64 * hd:64 * (hd + 1)],
                    in0=xr_sb[:R, 64 * hd:64 * (hd + 1)],
                    scalar=den[:, hd:hd + 1],
                    in1=ps_acc[:R, 64 * hd:64 * (hd + 1)],
                    op0=OP.mult, op1=OP.subtract)
            recm = work.tile([R, 2], F32, tag=tag + "_recm")
            nc.vector.reciprocal(out=recm[:], in_=den[:])
            ts_mul(recm[:], recm[:], -0.5)
            g = tabs.tile([R, 64], F32, tag=tag + "_g")
            r1 = work.tile([R, 64], F32, tag=tag + "_r1")
            ts_mul(g[:], nn[:, 0:64], recm[:, 0:1])
            ts_mul(r1[:], nn[:, 64:128], recm[:, 1:2])
            tt(out=g[:], in0=g[:], in1=r1[:], op=OP.add)
            tt(out=g[:], in0=g[:], in1=bias_rep[:R, :], op=OP.add)
            elu(g[:], R, tag + "_elu")
            return g

        # ---------------- minis ----------------
        def extract_mini(key, tag):
            ps = psum.tile([32, W], F32, tag="ps")
            gather_seq(ps[:], [(sv8[:, 32 * ch:32 * (ch + 1)],
                                vslice(VThi, key, ch), vslice(VTlo, key, ch))
                               for ch in range(2)])
            m_ = tabs.tile([32, W], F32, tag=tag)
            nc.vector.tensor_copy(out=m_[:], in_=ps[:])
            hi, lo = hilo(m_[:], tag, W)
            return m_, hi, lo

        XLsm, XLsm_hi, XLsm_lo = extract_mini("XLs", "XLsm")
        XRsm, XRsm_hi, XRsm_lo = extract_mini("XRs", "XRsm")
        dbg_dump("XLsm", XLsm[:])
        dbg_dump("XRsm", XRsm[:])

        # ---------------- B ----------------
        ps_bd = psacc.tile([32, 2 * W], F32, tag="ps_bd")
        ps_b = ps_bd[:, 0:W]
        ps_d = ps_bd[:, W:2 * W]
        u_self = tabs.tile([32, W], F32, tag="u_self")
        tt(out=u_self[:], in0=XLsm[:], in1=XRsm[:], op=OP.add)
        rhsS = edge_stage_small(u_self, 32, att1r, "bself")
        mm(ps_b, sdiag[:], rhsS[:], start=True, stop=False,
           skip_group_check=True)

        def b_gather(t, sl):
            pairs = [(bsrc8[:, (2 * t + ch) * 128:(2 * t + ch + 1) * 128],
                      vslice(VThi, "XL", ch), vslice(VTlo, "XL", ch))
                     for ch in range(2)]
            pairs.append((xr8[:, 128 * t:128 * (t + 1)], XRsm_hi[:],
                          XRsm_lo[:]))
            gather_seq(sl, pairs)

        def b_scatter(t, rhs_ap):
            mm(ps_b, bsc32[:, 32 * t:32 * (t + 1)], rhs_ap,
               start=False, stop=(t == EB - 1), skip_group_check=True)

        run_groups("B", EB, b_gather, b_scatter, att1r)
        if D.get("_dbg"):
            psbcp = work.tile([32, 2 * W], F32, tag="dbg_psb")
            nc.vector.tensor_copy(out=psbcp[:], in_=ps_bd[:])
            dbg_dump("ps_bd", psbcp[:])
        g1self = nd_post(ps_b, XRsm[:], g1bias, 32, "bpost")
        dbg_dump("g1self", g1self[:])

        # ---------------- XR2S / T2self / D-self (early) --------------
        ps = psum.tile([64, 32], F32, tag="ps")
        nc.tensor.transpose(ps[:], g1self[:], ident[:32, :32])
        g1sT = work.tile([64, 32], F32, tag="g1sT")
        nc.vector.tensor_copy(out=g1sT[:], in_=ps[:])
        ps2 = psum.tile([32, W], F32, tag="ps")
        mm(ps2[:], g1sT[:], w2ra[:], start=True, stop=True)
        XR2S = tabs.tile([32, W], F32, tag="XR2S")
        tt(out=XR2S[:], in0=ps2[:], in1=blra[:32, :], op=OP.add)
        dbg_dump("XR2S", XR2S[:])
        XR2S_hi, XR2S_lo = hilo(XR2S[:], "XR2S", W)
        ps3 = psum.tile([32, W], F32, tag="ps")
        mm(ps3[:], g1sT[:], w2la[:], start=True, stop=True)
        u_ds = tabs.tile([32, W], F32, tag="u_ds")
        tt(out=u_ds[:], in0=ps3[:], in1=XR2S[:], op=OP.add)
        rhsS2 = edge_stage_small(u_ds, 32, att2r, "dself")
        mm(ps_d, sdiag[:], rhsS2[:], start=True, stop=False,
           skip_group_check=True)

        # ---------------- P1 ----------------
        ps_num = psacc.tile([128, 2 * W], F32, tag="ps_num")
        dst_pos, pos = {}, 0
        for t in range(ET_P1):
            for c in dst_chunks[t]:
                dst_pos[(t, c)] = pos
                pos += 1
        sc_pos, pos = {}, 0
        for t in range(ET_P1):
            for h in sc_halves[t]:
                sc_pos[(t, h)] = pos
                pos += 1
        first_h = {h: min(t for t in range(ET_P1) if h in sc_halves[t])
                   for h in range(2)}
        last_h = {h: max(t for t in range(ET_P1) if h in sc_halves[t])
                  for h in range(2)}

        def p1_gather(t, sl):
            pairs = [(p1src8[:, (2 * t + ch) * 128:(2 * t + ch + 1) * 128],
                      vslice(VThi, "XL", ch), vslice(VTlo, "XL", ch))
                     for ch in range(2)]
            for c in dst_chunks[t]:
                j = dst_pos[(t, c)]
                pairs.append((p1dst8[:, 128 * j:128 * (j + 1)],
                              vslice(VThi, "XR", c), vslice(VTlo, "XR", c)))
            gather_seq(sl, pairs)

        def p1_scatter(t, rhs_ap):
            for h in sc_halves[t]:
                j = sc_pos[(t, h)]
                mm(ps_num[:, W * h:W * (h + 1)],
                   p1sc32[:, 128 * j:128 * (j + 1)], rhs_ap,
                   start=(t == first_h[h]), stop=(t == last_h[h]),
                   skip_group_check=True)

        run_groups("P1", ET_P1, p1_gather, p1_scatter, att1r)

        # ---------------- T1N assembly + hi/lo ------------------------
        T1N = tabs.tile([128, 2 * W], F32, tag="T1N")
        for h in range(2):
            pna = ps_num[:, W * h:W * (h + 1)]
            act(out=T1N[:, W * h + 128:W * h + 130], in_=pna[:, 128:130],
                func=AF.Copy)
            xroff = OFF["XR"] + W * h
            for hd in range(NH):
                stt(out=T1N[:, W * h + 64 * hd:W * h + 64 * (hd + 1)],
                    in0=VT[:, xroff + 64 * hd:xroff + 64 * (hd + 1)],
                    scalar=T1N[:, W * h + 128 + hd:W * h + 129 + hd],
                    in1=pna[:, 64 * hd:64 * (hd + 1)],
                    op0=OP.mult, op1=OP.subtract)
        if D.get("_dbg"):
            psncp = work.tile([128, 2 * W], F32, tag="dbg_psn")
            nc.vector.tensor_copy(out=psncp[:], in_=ps_num[:])
            dbg_dump("ps_num", psncp[:])
        dbg_dump("T1N", T1N[:])
        T1Nhi, T1Nlo = hilo(T1N[:], "T1N", 2 * W)

        # ---------------- A: gathers + u_n/u_o (prefix) ---------------
        ps_a1 = psum.tile([128, 2 * W], F32, tag="psA")  # xr_d | t1n_d
        gather_seq(ps_a1[:, 0:W],
                   [(a_d8[:, 128 * ch:128 * (ch + 1)],
                     vslice(VThi, "XR", ch), vslice(VTlo, "XR", ch))
                    for ch in range(2)])
        gather_seq(ps_a1[:, W:2 * W],
                   [(a_d8[:, 128 * ch:128 * (ch + 1)],
                     T1Nhi[:, W * ch:W * (ch + 1)],
                     T1Nlo[:, W * ch:W * (ch + 1)]) for ch in range(2)])
        ps_a2 = psum.tile([128, 2 * W], F32, tag="psA")  # xls_v | xl_v
        gather_seq(ps_a2[:, 0:W], [(a_xls8[:], XLsm_hi[:], XLsm_lo[:])])
        gather_seq(ps_a2[:, W:2 * W],
                   [(a_xl8[:, 128 * ch:128 * (ch + 1)],
                     vslice(VThi, "XL", ch), vslice(VTlo, "XL", ch))
                    for ch in range(2)])
        xr_d = work.tile([128, W], F32, tag="a_xrd")
        act(out=xr_d[:], in_=ps_a1[:, 0:W], func=AF.Copy)
        u_n = work.tile([128, W], F32, tag="a_un")
        tt(out=u_n[:], in0=ps_a2[:, 0:W], in1=xr_d[:], op=OP.add)
        u_o = work.tile([128, W], F32, tag="a_uo")
        tt(out=u_o[:], in0=ps_a2[:, W:2 * W], in1=xr_d[:], op=OP.add)

        # ---------------- g1 base + T2base + hi/lo --------------------
        T2B = tabs.tile([128, 2 * W], F32, tag="T2B")
        for h in range(2):
            recm = work.tile([128, 2], F32, tag="g1b_recm")
            nc.vector.reciprocal(out=recm[:],
                                 in_=T1N[:, W * h + 128:W * h + 130])
            ts_mul(recm[:], recm[:], -0.5)
            g1b = work.tile([128, 64], F32, tag="g1b_g")
            r1 = work.tile([128, 64], F32, tag="g1b_r1")
            ts_mul(g1b[:], T1N[:, W * h:W * h + 64], recm[:, 0:1])
            ts_mul(r1[:], T1N[:, W * h + 64:W * h + 128], recm[:, 1:2])
            tt(out=g1b[:], in0=g1b[:], in1=r1[:], op=OP.add)
            tt(out=g1b[:], in0=g1b[:], in1=g1bias[:], op=OP.add)
            elu(g1b[:], 128, "g1b_elu%d" % h)
            ps = psum.tile([64, 128], F32, tag="ps")
            nc.tensor.transpose(ps[:], g1b[:], ident[:])
            gT = work.tile([64, 128], F32, tag="g1bT")
            nc.vector.tensor_copy(out=gT[:], in_=ps[:])
            ps2 = psum.tile([128, W], F32, tag="ps")
            mm(ps2[:], gT[:], w2la[:], start=True, stop=True)
            nc.vector.tensor_copy(out=T2B[:, W * h:W * (h + 1)], in_=ps2[:])
        dbg_dump("T2B", T2B[:])
        T2Bhi, T2Blo = hilo(T2B[:], "T2B", 2 * W)

        # ---------------- D pass 1: T2base gathers --------------------
        D_TWO_PASS = False
        d_groups = []
        for g0 in range(0, EB, NG):
            ng = min(NG, EB - g0)
            ps_u = psum.tile([128, NG * W], F32, tag="psu")
            d_groups.append((g0, ng, ps_u))
            if not D_TWO_PASS:
                continue
            for i in range(ng):
                t = g0 + i
                sl = ps_u[:, W * i:W * (i + 1)]
                gather_seq(sl,
                           [(dsrc8[:, (3 * t + ch) * 128:
                                   (3 * t + ch + 1) * 128],
                             T2Bhi[:, W * ch:W * (ch + 1)],
                             T2Blo[:, W * ch:W * (ch + 1)])
                            for ch in range(2)], stop=False)

        # ---------------- A: edge stages + g1light + T2rare -----------
        def a_stage(u_sb, tag):
            absu = work.tile([128, 128], F32, tag=tag + "_absu")
            act(out=absu[:], in_=u_sb[:, 0:128], func=AF.Abs)
            tt(out=absu[:], in0=absu[:], in1=att1r[:], op=OP.mult)
            lgabs = work.tile([128, 2], F32, tag=tag + "_lg")
            red(out=lgabs[:], in_=absu[:].rearrange("p (h f) -> p h f", h=2),
                axis=AX.X, op=OP.add)
            wex = work.tile([128, 2], F32, tag=tag + "_w")
            stt(out=wex[:], in0=u_sb[:, 128:130], scalar=0.6, in1=lgabs[:],
                op0=OP.mult, op1=OP.add)
            act(out=wex[:], in_=wex[:], func=AF.Exp)
            ts_mul(wex[:], wex[:], a_C[:, 0:1])
            return wex

        wn = a_stage(u_n, "a_n")
        wo = a_stage(u_o, "a_o")

        nn_a = work.tile([128, 2, 64], F32, tag="a_nn")
        d_a = work.tile([128, 2, 64], F32, tag="a_d")
        tt(out=d_a[:], in0=ps_a2[:, W:W + 128].rearrange(
            "p (h f) -> p h f", h=2),
           in1=wo[:].rearrange("p h -> p h ()").to_broadcast([128, 2, 64]),
           op=OP.mult)
        tt(out=nn_a[:], in0=ps_a2[:, 0:128].rearrange(
            "p (h f) -> p h f", h=2),
           in1=wn[:].rearrange("p h -> p h ()").to_broadcast([128, 2, 64]),
           op=OP.mult)
        tt(out=d_a[:], in0=d_a[:], in1=nn_a[:], op=OP.subtract)
        tt(out=nn_a[:], in0=d_a[:],
           in1=ps_a1[:, W:W + 128].rearrange("p (h f) -> p h f", h=2),
           op=OP.add)
        den_a = work.tile([128, 2], F32, tag="a_den")
        tt(out=den_a[:], in0=wn[:], in1=wo[:], op=OP.subtract)
        tt(out=den_a[:], in0=den_a[:], in1=ps_a1[:, W + 128:W + 130],
           op=OP.add)
        nc.vector.tensor_scalar_max(out=den_a[:], in0=den_a[:],
                                    scalar1=1e-30)
        recm = work.tile([128, 2], F32, tag="a_recm")
        nc.vector.reciprocal(out=recm[:], in_=den_a[:])
        ts_mul(recm[:], recm[:], -0.5)
        g1light = tabs.tile([128, 64], F32, tag="g1light")
        r1a = work.tile([128, 64], F32, tag="a_r1")
        ts_mul(g1light[:], nn_a[:, 0, :], recm[:, 0:1])
        ts_mul(r1a[:], nn_a[:, 1, :], recm[:, 1:2])
        tt(out=g1light[:], in0=g1light[:], in1=r1a[:], op=OP.add)
        tt(out=g1light[:], in0=g1light[:], in1=g1bias[:], op=OP.add)
        elu(g1light[:], 128, "a_elu")

        ps = psum.tile([64, 128], F32, tag="ps")
        nc.tensor.transpose(ps[:], g1light[:], ident[:])
        gT = work.tile([64, 128], F32, tag="g1lT")
        nc.vector.tensor_copy(out=gT[:], in_=ps[:])
        ps2 = psum.tile([128, W], F32, tag="ps")
        mm(ps2[:], gT[:], w2la[:], start=True, stop=True)
        T2r = tabs.tile([128, W], F32, tag="T2r")
        nc.vector.tensor_copy(out=T2r[:], in_=ps2[:])
        dbg_dump("g1light", g1light[:])
        dbg_dump("T2r", T2r[:])
        T2r_hi, T2r_lo = hilo(T2r[:], "T2r", W)

        # ---------------- D pass 2: rare + xr gathers, stage, scatter --
        for (g0, ng, ps_u) in d_groups:
            for i in range(ng):
                t = g0 + i
                sl = ps_u[:, W * i:W * (i + 1)]
                pairs2 = [(dsrc8[:, (3 * t + 2) * 128:(3 * t + 3) * 128],
                           T2r_hi[:], T2r_lo[:]),
                          (xr8[:, 128 * t:128 * (t + 1)], XR2S_hi[:],
                           XR2S_lo[:])]
                if not D_TWO_PASS:
                    pairs2 = [(dsrc8[:, (3 * t + ch) * 128:
                                     (3 * t + ch + 1) * 128],
                               T2Bhi[:, W * ch:W * (ch + 1)],
                               T2Blo[:, W * ch:W * (ch + 1)])
                              for ch in range(2)] + pairs2
                gather_seq(sl, pairs2, start=not D_TWO_PASS)
            rhs = group_stage(ps_u, ng, "DD", att2r)
            for i in range(ng):
                t = g0 + i
                mm(ps_d, bsc32[:, 32 * t:32 * (t + 1)], rhs[:, i, :],
                   start=False, stop=(t == EB - 1), skip_group_check=True)

        if D.get("_dbg"):
            psdcp = work.tile([32, 2 * W], F32, tag="dbg_psd")
            nc.vector.tensor_copy(out=psdcp[:], in_=ps_bd[:])
            dbg_dump("ps_bd2", psdcp[:])
        g2 = nd_post(ps_d, XR2S[:], g2bias, 32, "dpost")
        dbg_dump("g2", g2[:])

        # ---------------- out (transposed; host untransposes) ---------
        ps = psum.tile([64, 32], F32, tag="ps")
        nc.tensor.transpose(ps[:], g2[:], ident[:32, :32])
        g2T = work.tile([64, 32], F32, tag="g2T")
        nc.vector.tensor_copy(out=g2T[:], in_=ps[:])
        pso = psum.tile([64, 32], F32, tag="ps")
        mm(pso[:], w_rec[:], g2T[:], start=True, stop=True)
        outT = work.tile([64, 32], F32, tag="outT")
        act(out=outT[:], in_=pso[:], func=AF.Tanh, bias=b_rec[:])
        dma(out=D["outT"][:], in_=outT[:])


# ------------------------------------------------------------------
# entry point
# ------------------------------------------------------------------

_CACHE = {}
TRACE = False
LAST_RESULT = None


def kernel(**inputs):
    global LAST_RESULT
    inputs = {k: np.asarray(v) for k, v in inputs.items()}
    shared, percore, dims = _build_tables(inputs["edge_index"])
    P = _prep_weights(inputs)
    key = (dims["EB"], dims["dst_chunks"], dims["sc_halves"])
    if key not in _CACHE:
        _CACHE[key] = _build_program(dims)
    nc = _CACHE[key]
    base = dict(P)
    base.update(shared)
    in_maps = []
    for c in range(NCORES):
        m = dict(base)
        m.update(percore[c])
        in_maps.append({k: np.ascontiguousarray(v) for k, v in m.items()})
    kw = {}
    if TRACE:
        kw = dict(trace=True, trace_cores=list(range(NCORES)))
    res = run_bass_kernel_spmd(nc, in_maps, core_ids=list(range(NCORES)),
                               **kw)
    LAST_RESULT = res
    out = np.concatenate([res.results[c]["outT"].T for c in range(NCORES)],
                         axis=0)
    return out.astype(np.float32)
